# revision 1
# baseline (speedup 1.0000x reference)
"""Trainium2 Bass kernel for masked-attention-like module:
    q = x@Wq; k = x@Wk; v = x@Wv
    scores = (q @ k.T) * tril(l)
    out = scores @ v
T=8192, D_IN=512, D_QK=D_V=64, fp32 inputs/outputs, 8 NeuronCores.

Strategy (sequence-parallel over T, load-balanced over the tril):
  - Work is tiled into [512 t x 512 s] macro-tiles of the lower triangle.
    Core c owns two t-panels: rows [512c, 512c+512) and
    [8192-512(c+1), 8192-512c).  That gives every core exactly 17
    macro-tiles -> identical, branch-free SPMD program.
  - Phase 1 (small SPMD kernel): each core computes qT/kT (fp16) and v
    (fp16) for its own 1024 rows from a host-pre-transposed x block.
  - Host gathers the tiny projections (pure data movement), then packs
    per-core, per-macro-tile operand arrays.  The l tile for each
    macro-tile is pre-masked (tril) and pre-transposed on the host, so
    the device computes scores directly in transposed layout:
        S^T[s,t] = sum_n kT[n,s] qT[n,t]   (PE, fp16)
        Sm^T = S^T * lT                    (DVE, fp32 l, fp16 out)
        outT[i] += v[s-chunk].T-free @ ...  (PE, fp16, accumulate)
    -> no on-chip transposes at all.
  - Phase 2 emits per-macro-tile partial outputs outT [64, 512]; the
    host sums partials per panel and transposes into the final [T, 64].
"""

import json

import numpy as np

T = 8192
D_IN = 512
D_QK = 64
D_V = 64
NCORES = 8
PANEL = 512  # rows per t-panel
NITEMS = 17  # macro-tiles per core

# ---------------------------------------------------------------------------
# Workaround: the walrus build in this container accepts only ONE sync-wait
# per instruction, but Tile attaches several (e.g. to the tail Drain).  Split
# multi-wait instructions at the BIR-JSON level by inserting single-wait NoOps
# on the same engine immediately before the instruction.
# ---------------------------------------------------------------------------
_fix_installed = [False]
_split_counter = [0]


def _fix_bir_json(bir_json):
    m = json.loads(bir_json)
    for f in m.get("functions", []):
        for blk in f.get("blocks", []):
            new_insts = []
            for inst in blk.get("instructions", []):
                si = inst.get("sync_info") or {}
                waits = si.get("on_wait") or []
                if len(waits) > 1:
                    for w in waits[:-1]:
                        _split_counter[0] += 1
                        new_insts.append({
                            "name": f"I-waitsplit-{_split_counter[0]}",
                            "opcode": "NoOp",
                            "engine": inst.get("engine"),
                            "ins": [],
                            "outs": [],
                            "sync_info": {"on_wait": [w], "on_update": []},
                        })
                    si = dict(si)
                    si["on_wait"] = waits[-1:]
                    inst = dict(inst)
                    inst["sync_info"] = si
                new_insts.append(inst)
            blk["instructions"] = new_insts
    return json.dumps(m).encode()


def _install_bir_fix():
    if _fix_installed[0]:
        return
    _fix_installed[0] = True
    import concourse.bass_utils as bu
    import concourse.bass2jax as b2j

    orig = bu.compile_bir_kernel

    def patched(bir_json, tmpdir, neff_name="file.neff"):
        return orig(_fix_bir_json(bir_json), tmpdir, neff_name)

    bu.compile_bir_kernel = patched
    b2j.compile_bir_kernel = patched


# ---------------------------------------------------------------------------
# Per-core work-item list: (t0, s0) macro-tile origins, 17 per core.
# ---------------------------------------------------------------------------
def _core_items(c):
    """17 macro-tiles: positions 0/1 are the two diagonal tiles (uniform
    across cores), positions 2..16 the fifteen strictly-lower full tiles."""
    tA = 512 * c
    tB = T - 512 * (c + 1)
    items = [(tA, tA), (tB, tB)]
    items += [(tA, 512 * j) for j in range(c)]
    items += [(tB, 512 * j) for j in range(15 - c)]
    assert len(items) == NITEMS
    return items


# ---------------------------------------------------------------------------
# Bass kernel builders
# ---------------------------------------------------------------------------
def _build_phase1():
    import concourse.bass as bass
    import concourse.mybir as mybir
    from concourse.tile import TileContext

    f32 = mybir.dt.float32
    f16 = mybir.dt.float16

    nc = bass.Bass(target_bir_lowering=False, trn_type="TRN2")
    # inputs: host-packed transposed x block + packed weights
    xTp = nc.dram_tensor("xTp", [128, 4 * 1024], f32, kind="ExternalInput")
    Wqp = nc.dram_tensor("Wqp", [128, 4 * 64], f32, kind="ExternalInput")
    Wkp = nc.dram_tensor("Wkp", [128, 4 * 64], f32, kind="ExternalInput")
    Wvp = nc.dram_tensor("Wvp", [128, 4 * 64], f32, kind="ExternalInput")
    qT_o = nc.dram_tensor("qT_o", [64, 1024], f16, kind="ExternalOutput")
    kT_o = nc.dram_tensor("kT_o", [64, 1024], f16, kind="ExternalOutput")
    v_o = nc.dram_tensor("v_o", [128, 8 * 64], f16, kind="ExternalOutput")

    with TileContext(nc) as tc:
        with (
            tc.tile_pool(name="sb", bufs=1) as sb,
            tc.tile_pool(name="xchunks", bufs=3) as xch,
            tc.tile_pool(name="outp", bufs=3) as outp,
            tc.tile_pool(name="ps", bufs=1, space="PSUM") as ps,
        ):
            wq = sb.tile([128, 256], f32, tag="wq")
            wk = sb.tile([128, 256], f32, tag="wk")
            wv = sb.tile([128, 256], f32, tag="wv")
            nc.scalar.dma_start(wq[:], Wqp[:])
            nc.scalar.dma_start(wk[:], Wkp[:])
            nc.scalar.dma_start(wv[:], Wvp[:])
            wq16 = sb.tile([128, 256], f16, tag="wq16")
            wk16 = sb.tile([128, 256], f16, tag="wk16")
            wv16 = sb.tile([128, 256], f16, tag="wv16")
            nc.vector.tensor_copy(wq16[:], wq[:])
            nc.vector.tensor_copy(wk16[:], wk[:])
            nc.vector.tensor_copy(wv16[:], wv[:])

            # qT/kT accumulators: one PSUM bank per accumulation group.
            pq = [
                ps.tile([64, 512], f32, tag=f"pq{j}", name=f"pq{j}")
                for j in range(4)
            ]

            # d-chunk pipeline: DMA chunk -> fp16 round -> 4 accumulating MMs;
            # all four fp16 chunks stay resident for the v pass below.
            xc16s = []
            for dc in range(4):
                xc = xch.tile([128, 1024], f32, tag="xc")
                nc.sync.dma_start(xc[:], xTp[:, dc * 1024:(dc + 1) * 1024])
                xc16 = xch.tile([128, 1024], f16, tag="xc16", bufs=4)
                nc.vector.tensor_copy(xc16[:], xc[:])
                xc16s.append(xc16)
                j = 0
                for w16 in (wq16, wk16):
                    for sbk in range(2):
                        nc.tensor.matmul(
                            pq[j][:],
                            w16[:, dc * 64:(dc + 1) * 64],
                            xc16[:, sbk * 512: sbk * 512 + 512],
                            start=(dc == 0),
                            stop=(dc == 3),
                        )
                        j += 1
            j = 0
            for dst in (qT_o, kT_o):
                for sbk in range(2):
                    ot = outp.tile([64, 512], f16, tag="po")
                    nc.vector.tensor_copy(ot[:], pq[j][:])
                    nc.sync.dma_start(dst[:, sbk * 512:(sbk + 1) * 512], ot[:])
                    j += 1
            # v: one PSUM group per s-chunk, rotating through 3 banks.
            vt = outp.tile([128, 512], f16, tag="pvo")
            for sc in range(8):
                pvt = ps.tile([128, 64], f32, tag="pv", bufs=3)
                for dc in range(4):
                    nc.tensor.matmul(
                        pvt[:],
                        xc16s[dc][:, sc * 128: sc * 128 + 128],
                        wv16[:, dc * 64:(dc + 1) * 64],
                        start=(dc == 0),
                        stop=(dc == 3),
                    )
                nc.vector.tensor_copy(vt[:, sc * 64:(sc + 1) * 64], pvt[:])
            nc.sync.dma_start(v_o[:], vt[:])
    return nc


def _build_phase2():
    import concourse.bass as bass
    import concourse.mybir as mybir
    from concourse.tile import TileContext

    f32 = mybir.dt.float32
    f16 = mybir.dt.float16

    nc = bass.Bass(target_bir_lowering=False, trn_type="TRN2")
    # items 0/1 are the diagonal tiles (dense-packed lower-tri chunks only),
    # items 2..16 the full lower tiles.
    lwd = nc.dram_tensor("lwd", [2, 128, 1280], f32, kind="ExternalInput")
    lwp = nc.dram_tensor("lwp", [NITEMS - 2, 128, 2048], f32, kind="ExternalInput")
    kqwp = nc.dram_tensor("kqwp", [NITEMS, 64, 1024], f16, kind="ExternalInput")
    vwp = nc.dram_tensor("vwp", [NITEMS, 128, 256], f16, kind="ExternalInput")
    po = nc.dram_tensor("po", [NITEMS, 64, 512], f16, kind="ExternalOutput")

    DIAG_OFF = [0, 512, 896, 1152]  # prefix sums of widths 512,384,256,128

    with TileContext(nc) as tc:
        with (
            tc.tile_pool(name="lw", bufs=4) as lwpool,
            tc.tile_pool(name="ops", bufs=4) as ops,
            tc.tile_pool(name="smt", bufs=8) as smtp,
            tc.tile_pool(name="osb", bufs=3) as osb,
            tc.tile_pool(name="psS", bufs=4, space="PSUM") as psS,
            tc.tile_pool(name="psO", bufs=2, space="PSUM") as psO,
        ):
            for i in range(NITEMS):
                diag = i < 2
                if diag:
                    lw = lwpool.tile([128, 1280], f32, tag="lwd", bufs=2)
                    nc.sync.dma_start(lw[:], lwd[i])
                else:
                    lw = lwpool.tile([128, 2048], f32, tag="lw", bufs=6)
                    nc.sync.dma_start(lw[:], lwp[i - 2])
                kq = ops.tile([64, 1024], f16, tag="kq")
                vw = ops.tile([128, 256], f16, tag="vw")
                nc.scalar.dma_start(kq[:], kqwp[i])
                nc.scalar.dma_start(vw[:], vwp[i])
                out_ps = psO.tile([64, 512], f32, tag="out")
                for sc in range(4):
                    if diag:
                        w = 512 - 128 * sc
                        t0, off = 128 * sc, DIAG_OFF[sc]
                    else:
                        w, t0, off = 512, 0, 512 * sc
                    s_ps = psS.tile([128, 512], f32, tag="S")
                    nc.tensor.matmul(
                        s_ps[:, :w],
                        kq[:, sc * 128:(sc + 1) * 128],
                        kq[:, 512 + t0:1024],
                        start=True,
                        stop=True,
                    )
                    smt = smtp.tile([128, 512], f16, tag="smt")
                    nc.vector.tensor_mul(
                        smt[:, :w], s_ps[:, :w], lw[:, off:off + w]
                    )
                    nc.tensor.matmul(
                        out_ps[:, t0:512],
                        vw[:, sc * 64:(sc + 1) * 64],
                        smt[:, :w],
                        start=(sc == 0),
                        stop=(sc == 3),
                    )
                ot = osb.tile([64, 512], f16, tag="ot")
                nc.scalar.copy(ot[:], out_ps[:])
                nc.scalar.dma_start(po[i], ot[:])
    return nc


_nc_cache = {}


def _get_nc(which):
    if which not in _nc_cache:
        _nc_cache[which] = _build_phase1() if which == 1 else _build_phase2()
    return _nc_cache[which]


# ---------------------------------------------------------------------------
# Host-side packing helpers (pure data movement)
# ---------------------------------------------------------------------------
def _pack_chunks(a, nchunk, rows):
    """[nchunk*rows, w] -> [rows, nchunk*w] with chunk-major free dim."""
    w = a.shape[1]
    return np.ascontiguousarray(
        a.reshape(nchunk, rows, w).transpose(1, 0, 2).reshape(rows, nchunk * w)
    )


def kernel(x, Wq, Wk, Wv, l):
    _install_bir_fix()
    from concourse import bass_utils

    x = np.asarray(x, dtype=np.float32)
    Wq = np.asarray(Wq, dtype=np.float32)
    Wk = np.asarray(Wk, dtype=np.float32)
    Wv = np.asarray(Wv, dtype=np.float32)
    l = np.asarray(l, dtype=np.float32)

    core_ids = list(range(NCORES))

    # ---------------- Phase 1: per-core projections -----------------------
    wq_p = _pack_chunks(Wq, 4, 128)
    wk_p = _pack_chunks(Wk, 4, 128)
    wv_p = _pack_chunks(Wv, 4, 128)
    in1 = []
    panels = []
    for c in range(NCORES):
        tA = 512 * c
        tB = T - 512 * (c + 1)
        panels.append((tA, tB))
        xcat = np.concatenate([x[tA:tA + 512], x[tB:tB + 512]], axis=0)  # [1024, 512]
        xT = np.ascontiguousarray(xcat.T)  # [512, 1024]
        xTp = _pack_chunks(xT, 4, 128)  # [128, 4096]
        in1.append({"xTp": xTp, "Wqp": wq_p, "Wkp": wk_p, "Wvp": wv_p})

    res1 = bass_utils.run_bass_kernel_spmd(_get_nc(1), in1, core_ids=core_ids)

    qT_full = np.empty((64, T), dtype=np.float16)
    kT_full = np.empty((64, T), dtype=np.float16)
    v_full = np.empty((T, 64), dtype=np.float16)
    for c in range(NCORES):
        tA, tB = panels[c]
        r = res1.results[c]
        qT_full[:, tA:tA + 512] = r["qT_o"][:, :512]
        qT_full[:, tB:tB + 512] = r["qT_o"][:, 512:]
        kT_full[:, tA:tA + 512] = r["kT_o"][:, :512]
        kT_full[:, tB:tB + 512] = r["kT_o"][:, 512:]
        vup = r["v_o"].reshape(128, 8, 64).transpose(1, 0, 2).reshape(1024, 64)
        v_full[tA:tA + 512] = vup[:512]
        v_full[tB:tB + 512] = vup[512:]

    # ---------------- Phase 2: masked scores + PV -------------------------
    in2 = []
    diag_off = [0, 512, 896, 1152]
    for c in range(NCORES):
        items = _core_items(c)
        lwd = np.empty((2, 128, 1280), dtype=np.float32)
        lwp = np.empty((NITEMS - 2, 128, 2048), dtype=np.float32)
        kqwp = np.empty((NITEMS, 64, 1024), dtype=np.float16)
        vwp = np.empty((NITEMS, 128, 256), dtype=np.float16)
        for i, (t0, s0) in enumerate(items):
            lt = l[t0:t0 + 512, s0:s0 + 512]
            if i < 2:
                lT = np.tril(lt).T  # [512 s, 512 t], upper-tri in (s,t)
                for sc in range(4):
                    w = 512 - 128 * sc
                    lwd[i][:, diag_off[sc]:diag_off[sc] + w] = (
                        lT[128 * sc:128 * (sc + 1), 128 * sc:512]
                    )
            else:
                lT = lt.T  # [512 s, 512 t]
                lwp[i - 2] = (
                    lT.reshape(4, 128, 512).transpose(1, 0, 2).reshape(128, 2048)
                )
            kqwp[i, :, :512] = kT_full[:, s0:s0 + 512]
            kqwp[i, :, 512:] = qT_full[:, t0:t0 + 512]
            vwp[i] = (
                v_full[s0:s0 + 512]
                .reshape(4, 128, 64)
                .transpose(1, 0, 2)
                .reshape(128, 256)
            )
        in2.append({"lwd": lwd, "lwp": lwp, "kqwp": kqwp, "vwp": vwp})

    res2 = bass_utils.run_bass_kernel_spmd(_get_nc(2), in2, core_ids=core_ids)

    out = np.empty((T, 64), dtype=np.float32)
    for c in range(NCORES):
        items = _core_items(c)
        tA, tB = panels[c]
        p = res2.results[c]["po"].astype(np.float32)  # [17, 64, 512]
        # item 0 = diag A, item 1 = diag B, 2..2+c-1 full A, rest full B
        pa = p[0] + p[2:2 + c].sum(axis=0)
        pb = p[1] + p[2 + c:].sum(axis=0)
        out[tA:tA + 512] = pa.T
        out[tB:tB + 512] = pb.T
    return out



# revision 39
# speedup vs baseline: 1.4994x; 1.4994x over previous
"""Trainium2 Bass kernel for masked-attention-like module:
    q = x@Wq; k = x@Wk; v = x@Wv
    scores = (q @ k.T) * tril(l)
    out = scores @ v
T=8192, D_IN=512, D_QK=D_V=64, fp32 inputs/outputs, 8 NeuronCores.

Strategy (sequence-parallel over T, load-balanced over the tril):
  - Work is tiled into [512 t x 512 s] macro-tiles of the lower triangle.
    Core c owns two t-panels: rows [512c, 512c+512) and
    [8192-512(c+1), 8192-512c).  That gives every core exactly 17
    macro-tiles -> identical, branch-free SPMD program.
  - All device operands are fp16, pre-packed on the host (pure data
    movement + dtype cast): l tiles are pre-masked (tril), transposed,
    and cast; x / W are cast for phase 1.
  - Phase 1 (small SPMD kernel): each core computes qT/kT and v (fp16)
    for its own 1024 rows from a host-pre-transposed fp16 x block.
  - Host gathers the tiny projections, then packs per-core per-item
    operands.  Phase 2 per macro-tile, all in transposed score layout:
        S^T[s,t] = sum_n kT[n,s] qT[n,t]      (PE, fp16)
        Sm^T = S^T * lT                       (DVE/Pool split, fp16 out)
        out[t,d] += Sm^T[s,t-chunk]^T-free @ v[s,d]  (PE, accumulate)
    The PV matmul uses Sm^T chunks as the stationary operand so the
    output lands in natural [t, d] layout with full 128-partition use
    (half the PE row count of the [d, t] formulation).
  - The score-mask multiplies are statically load-balanced between the
    DVE and Pool engines; the Activation engine drains PSUM->SBUF.
  - Per-item outputs are staged in SBUF and flushed with two batched
    DMAs; the host sums the per-panel partials (fp32).
"""

import json

import numpy as np

T = 8192
D_IN = 512
D_QK = 64
D_V = 64
NCORES = 8
PANEL = 512  # rows per t-panel
NITEMS = 17  # macro-tiles per core

# ---------------------------------------------------------------------------
# Workaround: the walrus build in this container accepts only ONE sync-wait
# per instruction, but Tile attaches several (e.g. to the tail Drain).  Split
# multi-wait instructions at the BIR-JSON level by inserting single-wait NoOps
# on the same engine immediately before the instruction.
# ---------------------------------------------------------------------------
_fix_installed = [False]
_split_counter = [0]


def _fix_bir_json(bir_json):
    m = json.loads(bir_json)
    for f in m.get("functions", []):
        for blk in f.get("blocks", []):
            new_insts = []
            for inst in blk.get("instructions", []):
                si = inst.get("sync_info") or {}
                waits = si.get("on_wait") or []
                if len(waits) > 1:
                    for w in waits[:-1]:
                        _split_counter[0] += 1
                        new_insts.append({
                            "name": f"I-waitsplit-{_split_counter[0]}",
                            "opcode": "NoOp",
                            "engine": inst.get("engine"),
                            "ins": [],
                            "outs": [],
                            "sync_info": {"on_wait": [w], "on_update": []},
                        })
                    si = dict(si)
                    si["on_wait"] = waits[-1:]
                    inst = dict(inst)
                    inst["sync_info"] = si
                new_insts.append(inst)
            blk["instructions"] = new_insts
    return json.dumps(m).encode()


def _install_bir_fix():
    if _fix_installed[0]:
        return
    _fix_installed[0] = True
    import concourse.bass_utils as bu
    import concourse.bass2jax as b2j

    orig = bu.compile_bir_kernel

    def patched(bir_json, tmpdir, neff_name="file.neff"):
        return orig(_fix_bir_json(bir_json), tmpdir, neff_name)

    bu.compile_bir_kernel = patched
    b2j.compile_bir_kernel = patched


# ---------------------------------------------------------------------------
# Per-core work-item list: (t0, s0) macro-tile origins, 17 per core.
# ---------------------------------------------------------------------------
def _core_items(c):
    """17 macro-tiles: positions 0/1 are the two diagonal tiles (uniform
    across cores), positions 2..16 the fifteen strictly-lower full tiles."""
    tA = 512 * c
    tB = T - 512 * (c + 1)
    items = [(tA, tA), (tB, tB)]
    items += [(tA, 512 * j) for j in range(c)]
    items += [(tB, 512 * j) for j in range(15 - c)]
    assert len(items) == NITEMS
    return items


# kq/vp DMA batching over items: 5 batches.
BATCHES = [(0, 2), (2, 6), (6, 10), (10, 14), (14, 17)]
DIAG_OFF = [0, 512, 896, 1152]  # prefix sums of widths 512,384,256,128

# PE p-state filler tuning (rows of scratch matmul work)
P1_FILL_PRE = 4
P2_FILL_PRE = 8
P2_FILL_STEP = 128
P2_MD = 3  # mult stage trails S stage
P2_SMT_BUFS = 16
P2_MP = 8  # PV stage trails S stage
# lw full-tile DMA batching (indices into the 15 full tiles)
LW_BATCH = [(0, 2), (2, 4), (4, 6), (6, 8), (8, 10), (10, 12), (12, 13),
            (13, 14), (14, 15)]


# ---------------------------------------------------------------------------
# Bass kernel builders
# ---------------------------------------------------------------------------
def _build_phase1():
    import concourse.bass as bass
    import concourse.mybir as mybir
    from concourse.tile import TileContext

    f32 = mybir.dt.float32
    f16 = mybir.dt.float16

    nc = bass.Bass(target_bir_lowering=False, trn_type="TRN2")
    # host-pre-transposed, fp16, d-chunk-major x block: [128 d, dc*1024 + t]
    xTp = nc.dram_tensor("xTp", [128, 4096], f16, kind="ExternalInput")
    # packed weights: wq chunks (4*64) | wk chunks | wv chunks
    Wp = nc.dram_tensor("Wp", [128, 768], f16, kind="ExternalInput")
    qk_o = nc.dram_tensor("qk_o", [64, 2048], f16, kind="ExternalOutput")
    v_o = nc.dram_tensor("v_o", [128, 512], f16, kind="ExternalOutput")

    with TileContext(nc) as tc:
        with (
            tc.tile_pool(name="sb", bufs=1) as sb,
            tc.tile_pool(name="ps", bufs=1, space="PSUM") as ps,
        ):
            w = sb.tile([128, 768], f16, tag="w")
            nc.scalar.dma_start(w[:], Wp[:])
            # d-chunked x DMAs so the PE can start after the first chunk
            xt = {}
            for dc in range(4):
                xc = sb.tile([128, 1024], f16, tag=f"x{dc}", name=f"x{dc}")
                nc.sync.dma_start(xc[:], xTp[:, dc * 1024:(dc + 1) * 1024])
                for h in range(2):
                    xt[(dc, h)] = xc[:, h * 512:(h + 1) * 512]

            # PE p-state priming: the tensor engine clock ramps only under
            # sustained execution, so burn the cold period on scratch
            # matmuls while the first DMAs are in flight; real matmuls then
            # run at full clock.
            scr = sb.tile([128, 512], f16, tag="scr")
            nc.vector.memset(scr[:], 0.0)
            psF = ps.tile([128, 512], f32, tag="psF", name="psF")

            def filler(rows):
                nc.tensor.matmul(
                    psF[:, :rows],
                    scr[:, :128],
                    scr[:, :rows],
                    start=True,
                    stop=True,
                )

            for _ in range(P1_FILL_PRE):
                filler(512)

            # qT/kT: 4 psum groups (q/k x 2 sbuf-bank halves of t),
            # dc-outer order: all groups touch chunk dc before dc+1, so the
            # PE never waits on a chunk that is still in flight.
            pq = [
                ps.tile([64, 512], f32, tag=f"pq{j}", name=f"pq{j}")
                for j in range(4)
            ]
            for dc in range(4):
                for h in range(2):
                    for qk in range(2):
                        j = qk * 2 + h
                        wbase = qk * 256  # 0 -> Wq, 256 -> Wk
                        nc.tensor.matmul(
                            pq[j][:],
                            w[:, wbase + dc * 64:wbase + (dc + 1) * 64],
                            xt[(dc, h)][:],
                            start=(dc == 0),
                            stop=(dc == 3),
                        )
            # copies split between ACT and DVE; each qk half is flushed as
            # soon as its two copies land, on the queue that will not block
            # anything behind it (qk halves on sync, v halves on scalar).
            qk_st = sb.tile([64, 2048], f16, tag="qkst")
            for j in range(4):
                if j % 2 == 0:
                    nc.scalar.copy(qk_st[:, j * 512:(j + 1) * 512], pq[j][:])
                else:
                    nc.vector.tensor_copy(
                        qk_st[:, j * 512:(j + 1) * 512], pq[j][:]
                    )
                if j == 1:
                    nc.sync.dma_start(qk_o[:, 0:1024], qk_st[:, 0:1024])
            nc.sync.dma_start(qk_o[:, 1024:2048], qk_st[:, 1024:2048])

            # v in natural [t, d] layout: stationary x chunks, moving Wv.
            # Two psum banks of 4 t-chunks each: bank B accumulates while
            # bank A is drained in one bulk copy, so the PE never waits.
            v_st = sb.tile([128, 512], f16, tag="vst")
            for half in range(2):
                pvh = ps.tile([128, 256], f32, tag=f"pv{half}",
                              name=f"pv{half}")
                for tq in range(4):
                    tcn = half * 4 + tq
                    for dc in range(4):
                        nc.tensor.matmul(
                            pvh[:, tq * 64:(tq + 1) * 64],
                            xt[(dc, half)][:, tq * 128:(tq + 1) * 128],
                            w[:, 512 + dc * 64:512 + (dc + 1) * 64],
                            start=(dc == 0),
                            stop=(dc == 3),
                        )
                if half == 0:
                    nc.vector.tensor_copy(v_st[:, 0:256], pvh[:])
                    nc.sync.dma_start(v_o[:, 0:256], v_st[:, 0:256])
                else:
                    nc.scalar.copy(v_st[:, 256:512], pvh[:])
            nc.sync.dma_start(v_o[:, 256:512], v_st[:, 256:512])
    return nc


def _build_phase2():
    import concourse.bass as bass
    import concourse.mybir as mybir
    from concourse.tile import TileContext

    f32 = mybir.dt.float32
    f16 = mybir.dt.float16

    nc = bass.Bass(target_bir_lowering=False, trn_type="TRN2")
    # diag tiles, dense-packed lower-tri chunks, both items side by side
    lwd = nc.dram_tensor("lwd", [128, 2560], f16, kind="ExternalInput")
    lwp = nc.dram_tensor("lwp", [128, 15 * 2048], f16, kind="ExternalInput")
    # per item: kT tile (512) | qT tile (512), item-major columns
    kqp = nc.dram_tensor("kqp", [64, NITEMS * 1024], f16, kind="ExternalInput")
    # per item: v tile packed [128 s, sc*64 + d], item-major columns
    vp = nc.dram_tensor("vp", [128, NITEMS * 256], f16, kind="ExternalInput")
    # per item: out packed [128 t, tc*64 + d], item-major columns
    po = nc.dram_tensor("po", [128, NITEMS * 256], f16, kind="ExternalOutput")

    # Static load-balancer for the mask multiplies.  GPSIMD cannot touch
    # PSUM on TRN2, so the two legal paths are:
    #   A: DVE multiplies straight from PSUM (fp32 rate)
    #   B: ACT copies PSUM -> SBUF fp16, DVE multiplies all-fp16 (2x rate)
    eng_t = {"d": 0.0, "a": 0.0}

    def pick_path(width):
        cA = width * 1.0417 + 125.0
        cBa = width * 0.833 + 230.0
        cBd = width * 0.52 + 60.0
        endA = max(eng_t["d"] + cA, eng_t["a"])
        endB = max(eng_t["d"] + cBd, eng_t["a"] + cBa)
        if endA <= endB:
            eng_t["d"] += cA
            return "A"
        eng_t["d"] += cBd
        eng_t["a"] += cBa
        return "B"

    with TileContext(nc) as tc:
        with (
            tc.tile_pool(name="lw", bufs=1) as lwpool,
            tc.tile_pool(name="ops", bufs=1) as ops,
            tc.tile_pool(name="smt", bufs=8) as smtp,
            tc.tile_pool(name="stage", bufs=1) as stg,
            tc.tile_pool(name="psS", bufs=3, space="PSUM") as psS,
            tc.tile_pool(name="psO", bufs=2, space="PSUM") as psO,
        ):
            # ---- input DMA plan: lw stream on sync, kq/vp stream on scalar
            lwdt = lwpool.tile([128, 2560], f16, tag="lwd")
            kqt, vpt = [], []
            for b, (i0, i1) in enumerate(BATCHES):
                nb = i1 - i0
                kqt.append(
                    ops.tile([64, nb * 1024], f16, tag=f"kq{b}", name=f"kq{b}")
                )
                vpt.append(
                    ops.tile([128, nb * 256], f16, tag=f"vp{b}", name=f"vp{b}")
                )
            lwbt = []
            for j, (j0, j1) in enumerate(LW_BATCH):
                lwbt.append(
                    lwpool.tile(
                        [128, (j1 - j0) * 2048], f16, tag=f"lwb{j}",
                        name=f"lwb{j}",
                    )
                )

            def lw_of(i):
                """SBUF slice holding l tile i (full tiles only, i >= 2)."""
                for j, (j0, j1) in enumerate(LW_BATCH):
                    if j0 <= i - 2 < j1:
                        return lwbt[j][:, (i - 2 - j0) * 2048:(i - 1 - j0) * 2048]
                raise AssertionError

            # Issue ALL input DMAs on the sync queue in exact consumption
            # order: the transfer engine serves one queue in program order,
            # so the kq/vp batches land just-in-time for the S stage without
            # preempting the lw stream that feeds the mask multiplies.
            def kv_batch(b):
                i0, i1 = BATCHES[b]
                nc.sync.dma_start(kqt[b][:], kqp[:, i0 * 1024:i1 * 1024])
                nc.sync.dma_start(vpt[b][:], vp[:, i0 * 256:i1 * 256])

            nc.sync.dma_start(lwdt[:], lwd[:])
            kv_batch(0)
            kv_batch(1)
            for j, (j0, j1) in enumerate(LW_BATCH):
                if j in (2, 4, 6):
                    kv_batch(2 + (j - 2) // 2)
                nc.sync.dma_start(lwbt[j][:], lwp[:, j0 * 2048:j1 * 2048])

            stA = stg.tile([128, 8 * 256], f16, tag="stA")
            stB = stg.tile([128, 9 * 256], f16, tag="stB")

            # PE p-state priming (see phase 1): scratch matmuls ramp the
            # clock before real work arrives and plug feed gaps after.
            scr = stg.tile([128, 512], f16, tag="scr")
            nc.vector.memset(scr[:], 0.0)
            psF = psO.tile([128, 512], f32, tag="psF", name="psF", bufs=1)

            def filler(rows):
                nc.tensor.matmul(
                    psF[:, :rows],
                    scr[:, :128],
                    scr[:, :rows],
                    start=True,
                    stop=True,
                )

            for _ in range(P2_FILL_PRE):
                filler(512)

            def batch_of(i):
                for b, (i0, i1) in enumerate(BATCHES):
                    if i0 <= i < i1:
                        return b, i - i0
                raise AssertionError

            def geom(i, sc):
                if i < 2:
                    return 512 - 128 * sc, 128 * sc, DIAG_OFF[sc]
                return 512, 0, 512 * sc

            # Software pipeline over 68 (item, sc) stages, three decoupled
            # stage trails: the S matmul for stage s runs at step s, its
            # mask multiply (DVE/Pool) at s+MD, its PV matmuls at s+MP.
            # The deep PV trail keeps the PE queue head from ever waiting
            # on a mult result; scratch fillers plug residual feed gaps so
            # the PE p-state stays at full clock.
            MD = P2_MD
            MP = P2_MP
            NST = NITEMS * 4
            s_tiles = {}
            c_tiles = {}
            m_tiles = {}
            paths = {}
            out_ps = None

            def stage_ops(s):
                i, sc = divmod(s, 4)
                b, bo = batch_of(i)
                return (
                    i,
                    sc,
                    kqt[b][:, bo * 1024:(bo + 1) * 1024],
                    vpt[b][:, bo * 256:(bo + 1) * 256],
                    lwdt[:, i * 1280:(i + 1) * 1280] if i < 2 else lw_of(i),
                )

            for step in range(NST + MP):
                if step < NST:
                    i, sc, kq, vw, lw = stage_ops(step)
                    w, t0, _ = geom(i, sc)
                    s_ps = psS.tile([128, 512], f32, tag="S", bufs=MD + 2)
                    nc.tensor.matmul(
                        s_ps[:, :w],
                        kq[:, sc * 128:(sc + 1) * 128],
                        kq[:, 512 + t0:1024],
                        start=True,
                        stop=True,
                    )
                    s_tiles[step] = s_ps
                if MD <= step < NST + MD:
                    m = step - MD
                    i, sc, kq, vw, lw = stage_ops(m)
                    w, t0, off = geom(i, sc)
                    s_ps = s_tiles.pop(m)
                    smt = smtp.tile([128, 512], f16, tag="smt", bufs=P2_SMT_BUFS)
                    paths[m] = pick_path(w)
                    if paths[m] == "A":
                        nc.vector.tensor_mul(
                            smt[:, :w], s_ps[:, :w], lw[:, off:off + w]
                        )
                    else:
                        sc16 = smtp.tile([128, 512], f16, tag="sc16", bufs=6)
                        nc.scalar.copy(sc16[:, :w], s_ps[:, :w])
                        c_tiles[m] = (sc16, lw, off, w)
                    m_tiles[m] = smt
                if MD + 2 <= step < NST + MD + 2:
                    m = step - MD - 2
                    if paths[m] == "B":
                        sc16, lw, off, w = c_tiles.pop(m)
                        smt = m_tiles[m]
                        nc.vector.tensor_mul(
                            smt[:, :w], sc16[:, :w], lw[:, off:off + w]
                        )
                if step >= MP and (step - MP) % 4 == 3:
                    # whole-item PV block, tcn-major so each psum
                    # accumulation group closes before the next opens
                    i = (step - MP) // 4
                    _, _, kq, vw, lw = stage_ops(4 * i)
                    diag = i < 2
                    out_ps = psO.tile([128, 256], f32, tag="out")
                    smts = [m_tiles.pop(4 * i + sc) for sc in range(4)]
                    # PV: stationary = Sm^T t-chunk, moving = v chunk ->
                    # out[t, d] accumulated over s-chunks
                    for tcn in range(4):
                        for sc in range(0, (tcn + 1) if diag else 4):
                            col0 = (tcn - sc) * 128 if diag else tcn * 128
                            nc.tensor.matmul(
                                out_ps[:, tcn * 64:(tcn + 1) * 64],
                                smts[sc][:, col0:col0 + 128],
                                vw[:, sc * 64:(sc + 1) * 64],
                                start=(sc == 0),
                                stop=(sc == (tcn if diag else 3)),
                            )
                    if True:
                        st, o0 = (stA, i) if i < 8 else (stB, i - 8)
                        if eng_t["a"] <= eng_t["d"]:
                            eng_t["a"] += 360.0
                            nc.scalar.copy(
                                st[:, o0 * 256:(o0 + 1) * 256], out_ps[:]
                            )
                        else:
                            eng_t["d"] += 400.0
                            nc.vector.tensor_copy(
                                st[:, o0 * 256:(o0 + 1) * 256], out_ps[:]
                            )
                        if i < 8:
                            if i == 5:
                                nc.sync.dma_start(po[:, 0:6 * 256],
                                                  stA[:, 0:6 * 256])
                            if i == 7:
                                nc.sync.dma_start(po[:, 6 * 256:8 * 256],
                                                  stA[:, 6 * 256:])
                        else:
                            j = i - 8
                            if j == 5:
                                nc.sync.dma_start(
                                    po[:, 8 * 256:14 * 256], stB[:, 0:6 * 256]
                                )
                if step < NST and P2_FILL_STEP:
                    filler(P2_FILL_STEP)
            nc.sync.dma_start(po[:, 14 * 256:NITEMS * 256],
                              stB[:, 6 * 256:])
    return nc


_nc_cache = {}


def _get_nc(which):
    if which not in _nc_cache:
        _nc_cache[which] = _build_phase1() if which == 1 else _build_phase2()
    return _nc_cache[which]


# ---------------------------------------------------------------------------
# Host-side packing helpers (pure data movement + dtype cast)
# ---------------------------------------------------------------------------
def _pack_chunks(a, nchunk, rows):
    """[nchunk*rows, w] -> [rows, nchunk*w] with chunk-major free dim."""
    w = a.shape[1]
    return np.ascontiguousarray(
        a.reshape(nchunk, rows, w).transpose(1, 0, 2).reshape(rows, nchunk * w)
    )


def kernel(x, Wq, Wk, Wv, l):
    _install_bir_fix()
    from concourse import bass_utils

    x = np.asarray(x, dtype=np.float32)
    l = np.asarray(l, dtype=np.float32)

    core_ids = list(range(NCORES))

    # ---------------- Phase 1: per-core projections -----------------------
    Wp = np.concatenate(
        [
            _pack_chunks(np.asarray(wm, dtype=np.float16), 4, 128)
            for wm in (Wq, Wk, Wv)
        ],
        axis=1,
    )  # [128, 768]
    in1 = []
    panels = []
    for c in range(NCORES):
        tA = 512 * c
        tB = T - 512 * (c + 1)
        panels.append((tA, tB))
        xcat = np.concatenate([x[tA:tA + 512], x[tB:tB + 512]], axis=0)
        xT = np.ascontiguousarray(xcat.T).astype(np.float16)  # [512, 1024]
        xTp = _pack_chunks(xT, 4, 128)  # [128, 4096]
        in1.append({"xTp": xTp, "Wp": Wp})

    res1 = bass_utils.run_bass_kernel_spmd(_get_nc(1), in1, core_ids=core_ids)

    qT_full = np.empty((64, T), dtype=np.float16)
    kT_full = np.empty((64, T), dtype=np.float16)
    v_full = np.empty((T, 64), dtype=np.float16)
    for c in range(NCORES):
        tA, tB = panels[c]
        r = res1.results[c]
        qk = r["qk_o"]  # [64, 2048]: qA | qB | kA | kB
        qT_full[:, tA:tA + 512] = qk[:, 0:512]
        qT_full[:, tB:tB + 512] = qk[:, 512:1024]
        kT_full[:, tA:tA + 512] = qk[:, 1024:1536]
        kT_full[:, tB:tB + 512] = qk[:, 1536:2048]
        vup = r["v_o"].reshape(128, 8, 64).transpose(1, 0, 2).reshape(1024, 64)
        v_full[tA:tA + 512] = vup[:512]
        v_full[tB:tB + 512] = vup[512:]

    # ---------------- Phase 2: masked scores + PV -------------------------
    in2 = []
    for c in range(NCORES):
        items = _core_items(c)
        lwd = np.empty((128, 2560), dtype=np.float16)
        lwp = np.empty((128, 15 * 2048), dtype=np.float16)
        kqp = np.empty((64, NITEMS * 1024), dtype=np.float16)
        vpk = np.empty((128, NITEMS * 256), dtype=np.float16)
        for i, (t0, s0) in enumerate(items):
            lt = l[t0:t0 + 512, s0:s0 + 512]
            if i < 2:
                lT = np.tril(lt).T.astype(np.float16)  # [512 s, 512 t]
                for sc in range(4):
                    w = 512 - 128 * sc
                    o = i * 1280 + DIAG_OFF[sc]
                    lwd[:, o:o + w] = lT[128 * sc:128 * (sc + 1), 128 * sc:512]
            else:
                lT = lt.T.astype(np.float16)
                lwp[:, (i - 2) * 2048:(i - 1) * 2048] = (
                    lT.reshape(4, 128, 512).transpose(1, 0, 2).reshape(128, 2048)
                )
            kqp[:, i * 1024:i * 1024 + 512] = kT_full[:, s0:s0 + 512]
            kqp[:, i * 1024 + 512:(i + 1) * 1024] = qT_full[:, t0:t0 + 512]
            vpk[:, i * 256:(i + 1) * 256] = (
                v_full[s0:s0 + 512]
                .reshape(4, 128, 64)
                .transpose(1, 0, 2)
                .reshape(128, 256)
            )
        in2.append({"lwd": lwd, "lwp": lwp, "kqp": kqp, "vp": vpk})

    res2 = bass_utils.run_bass_kernel_spmd(_get_nc(2), in2, core_ids=core_ids)

    out = np.empty((T, 64), dtype=np.float32)
    for c in range(NCORES):
        tA, tB = panels[c]
        p = res2.results[c]["po"].astype(np.float32)  # [128, 17*256]
        # unpack per item: [128 t, tc*64 + d] -> [512 t, 64 d]
        pit = (
            p.reshape(128, NITEMS, 4, 64)
            .transpose(1, 2, 0, 3)
            .reshape(NITEMS, 512, 64)
        )
        # item 0 = diag A, item 1 = diag B, 2..2+c full A, rest full B
        out[tA:tA + 512] = pit[0] + pit[2:2 + c].sum(axis=0)
        out[tB:tB + 512] = pit[1] + pit[2 + c:].sum(axis=0)
    return out


# revision 42
# speedup vs baseline: 1.5968x; 1.0650x over previous
"""Trainium2 Bass kernel for masked-attention-like module:
    q = x@Wq; k = x@Wk; v = x@Wv
    scores = (q @ k.T) * tril(l)
    out = scores @ v
T=8192, D_IN=512, D_QK=D_V=64, fp32 inputs/outputs, 8 NeuronCores.

Strategy (sequence-parallel over T, load-balanced over the tril):
  - Work is tiled into [512 t x 512 s] macro-tiles of the lower triangle.
    Core c owns two t-panels: rows [512c, 512c+512) and
    [8192-512(c+1), 8192-512c).  That gives every core exactly 17
    macro-tiles -> identical, branch-free SPMD program.
  - All device operands are fp16, pre-packed on the host (pure data
    movement + dtype cast): l tiles are pre-masked (tril), transposed,
    and cast; x / W are cast for phase 1.
  - Phase 1 (small SPMD kernel): each core computes qT/kT and v (fp16)
    for its own 1024 rows from a host-pre-transposed fp16 x block.
  - Host gathers the tiny projections, then packs per-core per-item
    operands.  Phase 2 per macro-tile, all in transposed score layout:
        S^T[s,t] = sum_n kT[n,s] qT[n,t]      (PE, fp16)
        Sm^T = S^T * lT                       (DVE/Pool split, fp16 out)
        out[t,d] += Sm^T[s,t-chunk]^T-free @ v[s,d]  (PE, accumulate)
    The PV matmul uses Sm^T chunks as the stationary operand so the
    output lands in natural [t, d] layout with full 128-partition use
    (half the PE row count of the [d, t] formulation).
  - The score-mask multiplies are statically load-balanced between a
    direct DVE fp32 path and an ACT-copy + DVE fp16 (2x mode) path
    (GPSIMD cannot read PSUM on TRN2).  Scratch matmuls keep the PE
    p-state at full clock while real work streams in.
  - Per-item outputs are staged in SBUF and flushed with two batched
    DMAs; the host sums the per-panel partials (fp32).
"""

import json

import numpy as np

T = 8192
D_IN = 512
D_QK = 64
D_V = 64
NCORES = 8
PANEL = 512  # rows per t-panel
NITEMS = 17  # macro-tiles per core

# ---------------------------------------------------------------------------
# Workaround: the walrus build in this container accepts only ONE sync-wait
# per instruction, but Tile attaches several (e.g. to the tail Drain).  Split
# multi-wait instructions at the BIR-JSON level by inserting single-wait NoOps
# on the same engine immediately before the instruction.
# ---------------------------------------------------------------------------
_fix_installed = [False]
_split_counter = [0]


def _fix_bir_json(bir_json):
    m = json.loads(bir_json)
    for f in m.get("functions", []):
        for blk in f.get("blocks", []):
            new_insts = []
            for inst in blk.get("instructions", []):
                si = inst.get("sync_info") or {}
                waits = si.get("on_wait") or []
                if len(waits) > 1:
                    for w in waits[:-1]:
                        _split_counter[0] += 1
                        new_insts.append({
                            "name": f"I-waitsplit-{_split_counter[0]}",
                            "opcode": "NoOp",
                            "engine": inst.get("engine"),
                            "ins": [],
                            "outs": [],
                            "sync_info": {"on_wait": [w], "on_update": []},
                        })
                    si = dict(si)
                    si["on_wait"] = waits[-1:]
                    inst = dict(inst)
                    inst["sync_info"] = si
                new_insts.append(inst)
            blk["instructions"] = new_insts
    return json.dumps(m).encode()


def _install_bir_fix():
    if _fix_installed[0]:
        return
    _fix_installed[0] = True
    import concourse.bass_utils as bu
    import concourse.bass2jax as b2j

    orig = bu.compile_bir_kernel

    def patched(bir_json, tmpdir, neff_name="file.neff"):
        return orig(_fix_bir_json(bir_json), tmpdir, neff_name)

    bu.compile_bir_kernel = patched
    b2j.compile_bir_kernel = patched


# ---------------------------------------------------------------------------
# Per-core work-item list: (t0, s0) macro-tile origins, 17 per core.
# ---------------------------------------------------------------------------
def _core_items(c):
    """17 macro-tiles: positions 0/1 are the two diagonal tiles (uniform
    across cores), positions 2..16 the fifteen strictly-lower full tiles."""
    tA = 512 * c
    tB = T - 512 * (c + 1)
    items = [(tA, tA), (tB, tB)]
    items += [(tA, 512 * j) for j in range(c)]
    items += [(tB, 512 * j) for j in range(15 - c)]
    assert len(items) == NITEMS
    return items


# kq/vp DMA batching over items: 5 batches.
BATCHES = [(0, 2), (2, 6), (6, 10), (10, 14), (14, 17)]
DIAG_OFF = [0, 512, 896, 1152]  # prefix sums of widths 512,384,256,128

# PE p-state filler tuning (rows of scratch matmul work)
P1_FILL_PRE = 4
P2_FILL_PRE = 6
P2_FILL_STEP = 64
P2_MD = 3  # mult stage trails S stage
P2_SMT_BUFS = 16
P2_MP = 9  # PV stage trails S stage
# lw full-tile DMA batching (indices into the 15 full tiles)
LW_BATCH = [(0, 2), (2, 4), (4, 6), (6, 8), (8, 10), (10, 12), (12, 13),
            (13, 14), (14, 15)]


# ---------------------------------------------------------------------------
# Bass kernel builders
# ---------------------------------------------------------------------------
def _build_phase1():
    import concourse.bass as bass
    import concourse.mybir as mybir
    from concourse.tile import TileContext

    f32 = mybir.dt.float32
    f16 = mybir.dt.float16

    nc = bass.Bass(target_bir_lowering=False, trn_type="TRN2")
    # host-pre-transposed, fp16, d-chunk-major x block: [128 d, dc*1024 + t]
    xTp = nc.dram_tensor("xTp", [128, 4096], f16, kind="ExternalInput")
    # packed weights: wq chunks (4*64) | wk chunks | wv chunks
    Wp = nc.dram_tensor("Wp", [128, 768], f16, kind="ExternalInput")
    qk_o = nc.dram_tensor("qk_o", [64, 2048], f16, kind="ExternalOutput")
    v_o = nc.dram_tensor("v_o", [128, 512], f16, kind="ExternalOutput")

    with TileContext(nc) as tc:
        with (
            tc.tile_pool(name="sb", bufs=1) as sb,
            tc.tile_pool(name="ps", bufs=1, space="PSUM") as ps,
        ):
            w = sb.tile([128, 768], f16, tag="w")
            nc.scalar.dma_start(w[:], Wp[:])
            # d-chunked x DMAs so the PE can start after the first chunk
            xt = {}
            for dc in range(4):
                xc = sb.tile([128, 1024], f16, tag=f"x{dc}", name=f"x{dc}")
                nc.sync.dma_start(xc[:], xTp[:, dc * 1024:(dc + 1) * 1024])
                for h in range(2):
                    xt[(dc, h)] = xc[:, h * 512:(h + 1) * 512]

            # PE p-state priming: the tensor engine clock ramps only under
            # sustained execution, so burn the cold period on scratch
            # matmuls while the first DMAs are in flight; real matmuls then
            # run at full clock.
            scr = sb.tile([128, 512], f16, tag="scr")
            nc.vector.memset(scr[:], 0.0)
            psF = ps.tile([128, 512], f32, tag="psF", name="psF")

            def filler(rows):
                nc.tensor.matmul(
                    psF[:, :rows],
                    scr[:, :128],
                    scr[:, :rows],
                    start=True,
                    stop=True,
                )

            for _ in range(P1_FILL_PRE):
                filler(512)

            # qT/kT: 4 psum groups (q/k x 2 sbuf-bank halves of t),
            # dc-outer order: all groups touch chunk dc before dc+1, so the
            # PE never waits on a chunk that is still in flight.
            pq = [
                ps.tile([64, 512], f32, tag=f"pq{j}", name=f"pq{j}")
                for j in range(4)
            ]
            for dc in range(4):
                for h in range(2):
                    for qk in range(2):
                        j = qk * 2 + h
                        wbase = qk * 256  # 0 -> Wq, 256 -> Wk
                        nc.tensor.matmul(
                            pq[j][:],
                            w[:, wbase + dc * 64:wbase + (dc + 1) * 64],
                            xt[(dc, h)][:],
                            start=(dc == 0),
                            stop=(dc == 3),
                        )
            # copies split between ACT and DVE; each qk half is flushed as
            # soon as its two copies land, on the queue that will not block
            # anything behind it (qk halves on sync, v halves on scalar).
            qk_st = sb.tile([64, 2048], f16, tag="qkst")
            for j in range(4):
                if j % 2 == 0:
                    nc.scalar.copy(qk_st[:, j * 512:(j + 1) * 512], pq[j][:])
                else:
                    nc.vector.tensor_copy(
                        qk_st[:, j * 512:(j + 1) * 512], pq[j][:]
                    )
                if j == 1:
                    nc.sync.dma_start(qk_o[:, 0:1024], qk_st[:, 0:1024])
            nc.sync.dma_start(qk_o[:, 1024:2048], qk_st[:, 1024:2048])

            # v in natural [t, d] layout: stationary x chunks, moving Wv.
            # Two psum banks of 4 t-chunks each: bank B accumulates while
            # bank A is drained in one bulk copy, so the PE never waits.
            v_st = sb.tile([128, 512], f16, tag="vst")
            for half in range(2):
                pvh = ps.tile([128, 256], f32, tag=f"pv{half}",
                              name=f"pv{half}")
                for tq in range(4):
                    tcn = half * 4 + tq
                    for dc in range(4):
                        nc.tensor.matmul(
                            pvh[:, tq * 64:(tq + 1) * 64],
                            xt[(dc, half)][:, tq * 128:(tq + 1) * 128],
                            w[:, 512 + dc * 64:512 + (dc + 1) * 64],
                            start=(dc == 0),
                            stop=(dc == 3),
                        )
                if half == 0:
                    nc.vector.tensor_copy(v_st[:, 0:256], pvh[:])
                    nc.sync.dma_start(v_o[:, 0:256], v_st[:, 0:256])
                else:
                    nc.scalar.copy(v_st[:, 256:512], pvh[:])
            nc.sync.dma_start(v_o[:, 256:512], v_st[:, 256:512])
    return nc


def _build_phase2():
    import concourse.bass as bass
    import concourse.mybir as mybir
    from concourse.tile import TileContext

    f32 = mybir.dt.float32
    f16 = mybir.dt.float16

    nc = bass.Bass(target_bir_lowering=False, trn_type="TRN2")
    # diag tiles, dense-packed lower-tri chunks, both items side by side
    lwd = nc.dram_tensor("lwd", [128, 2560], f16, kind="ExternalInput")
    lwp = nc.dram_tensor("lwp", [128, 15 * 2048], f16, kind="ExternalInput")
    # per item: kT tile (512) | qT tile (512), item-major columns
    kqp = nc.dram_tensor("kqp", [64, NITEMS * 1024], f16, kind="ExternalInput")
    # per item: v tile packed [128 s, sc*64 + d], item-major columns
    vp = nc.dram_tensor("vp", [128, NITEMS * 256], f16, kind="ExternalInput")
    # per item: out packed [128 t, tc*64 + d], item-major columns
    po = nc.dram_tensor("po", [128, NITEMS * 256], f16, kind="ExternalOutput")

    # Static load-balancer for the mask multiplies.  GPSIMD cannot touch
    # PSUM on TRN2, so the two legal paths are:
    #   A: DVE multiplies straight from PSUM (fp32 rate)
    #   B: ACT copies PSUM -> SBUF fp16, DVE multiplies all-fp16 (2x rate)
    eng_t = {"d": 0.0, "a": 0.0}

    def pick_path(width):
        cA = width * 1.0417 + 125.0
        cBa = width * 0.833 + 230.0
        cBd = width * 0.52 + 60.0
        endA = max(eng_t["d"] + cA, eng_t["a"])
        endB = max(eng_t["d"] + cBd, eng_t["a"] + cBa)
        if endA <= endB:
            eng_t["d"] += cA
            return "A"
        eng_t["d"] += cBd
        eng_t["a"] += cBa
        return "B"

    with TileContext(nc) as tc:
        with (
            tc.tile_pool(name="lw", bufs=1) as lwpool,
            tc.tile_pool(name="ops", bufs=1) as ops,
            tc.tile_pool(name="smt", bufs=8) as smtp,
            tc.tile_pool(name="stage", bufs=1) as stg,
            tc.tile_pool(name="psS", bufs=3, space="PSUM") as psS,
            tc.tile_pool(name="psO", bufs=2, space="PSUM") as psO,
        ):
            # ---- input DMA plan: lw stream on sync, kq/vp stream on scalar
            lwdt = lwpool.tile([128, 2560], f16, tag="lwd")
            kqt, vpt = [], []
            for b, (i0, i1) in enumerate(BATCHES):
                nb = i1 - i0
                kqt.append(
                    ops.tile([64, nb * 1024], f16, tag=f"kq{b}", name=f"kq{b}")
                )
                vpt.append(
                    ops.tile([128, nb * 256], f16, tag=f"vp{b}", name=f"vp{b}")
                )
            lwbt = []
            for j, (j0, j1) in enumerate(LW_BATCH):
                lwbt.append(
                    lwpool.tile(
                        [128, (j1 - j0) * 2048], f16, tag=f"lwb{j}",
                        name=f"lwb{j}",
                    )
                )

            def lw_of(i):
                """SBUF slice holding l tile i (full tiles only, i >= 2)."""
                for j, (j0, j1) in enumerate(LW_BATCH):
                    if j0 <= i - 2 < j1:
                        return lwbt[j][:, (i - 2 - j0) * 2048:(i - 1 - j0) * 2048]
                raise AssertionError

            # Issue ALL input DMAs on the sync queue in exact consumption
            # order: the transfer engine serves one queue in program order,
            # so the kq/vp batches land just-in-time for the S stage without
            # preempting the lw stream that feeds the mask multiplies.
            def kv_batch(b):
                i0, i1 = BATCHES[b]
                nc.sync.dma_start(kqt[b][:], kqp[:, i0 * 1024:i1 * 1024])
                nc.sync.dma_start(vpt[b][:], vp[:, i0 * 256:i1 * 256])

            nc.sync.dma_start(lwdt[:], lwd[:])
            kv_batch(0)
            kv_batch(1)
            for j, (j0, j1) in enumerate(LW_BATCH):
                if j in (2, 4, 6):
                    kv_batch(2 + (j - 2) // 2)
                nc.sync.dma_start(lwbt[j][:], lwp[:, j0 * 2048:j1 * 2048])

            stA = stg.tile([128, 8 * 256], f16, tag="stA")
            stB = stg.tile([128, 9 * 256], f16, tag="stB")

            # PE p-state priming (see phase 1): scratch matmuls ramp the
            # clock before real work arrives and plug feed gaps after.
            scr = stg.tile([128, 512], f16, tag="scr")
            nc.vector.memset(scr[:], 0.0)
            psF = psO.tile([128, 512], f32, tag="psF", name="psF", bufs=1)

            def filler(rows):
                nc.tensor.matmul(
                    psF[:, :rows],
                    scr[:, :128],
                    scr[:, :rows],
                    start=True,
                    stop=True,
                )

            for _ in range(P2_FILL_PRE):
                filler(512)

            def batch_of(i):
                for b, (i0, i1) in enumerate(BATCHES):
                    if i0 <= i < i1:
                        return b, i - i0
                raise AssertionError

            def geom(i, sc):
                if i < 2:
                    return 512 - 128 * sc, 128 * sc, DIAG_OFF[sc]
                return 512, 0, 512 * sc

            # Software pipeline over 68 (item, sc) stages, three decoupled
            # stage trails: the S matmul for stage s runs at step s, its
            # mask multiply (DVE/Pool) at s+MD, its PV matmuls at s+MP.
            # The deep PV trail keeps the PE queue head from ever waiting
            # on a mult result; scratch fillers plug residual feed gaps so
            # the PE p-state stays at full clock.
            MD = P2_MD
            MP = P2_MP
            NST = NITEMS * 4
            s_tiles = {}
            c_tiles = {}
            m_tiles = {}
            paths = {}
            out_ps = None

            def stage_ops(s):
                i, sc = divmod(s, 4)
                b, bo = batch_of(i)
                return (
                    i,
                    sc,
                    kqt[b][:, bo * 1024:(bo + 1) * 1024],
                    vpt[b][:, bo * 256:(bo + 1) * 256],
                    lwdt[:, i * 1280:(i + 1) * 1280] if i < 2 else lw_of(i),
                )

            for step in range(NST + MP):
                if step < NST:
                    i, sc, kq, vw, lw = stage_ops(step)
                    w, t0, _ = geom(i, sc)
                    s_ps = psS.tile([128, 512], f32, tag="S", bufs=MD + 2)
                    nc.tensor.matmul(
                        s_ps[:, :w],
                        kq[:, sc * 128:(sc + 1) * 128],
                        kq[:, 512 + t0:1024],
                        start=True,
                        stop=True,
                    )
                    s_tiles[step] = s_ps
                if MD <= step < NST + MD:
                    m = step - MD
                    i, sc, kq, vw, lw = stage_ops(m)
                    w, t0, off = geom(i, sc)
                    s_ps = s_tiles.pop(m)
                    smt = smtp.tile([128, 512], f16, tag="smt", bufs=P2_SMT_BUFS)
                    paths[m] = pick_path(w)
                    if paths[m] == "A":
                        nc.vector.tensor_mul(
                            smt[:, :w], s_ps[:, :w], lw[:, off:off + w]
                        )
                    else:
                        sc16 = smtp.tile([128, 512], f16, tag="sc16", bufs=6)
                        nc.scalar.copy(sc16[:, :w], s_ps[:, :w])
                        c_tiles[m] = (sc16, lw, off, w)
                    m_tiles[m] = smt
                if MD + 2 <= step < NST + MD + 2:
                    m = step - MD - 2
                    if paths[m] == "B":
                        sc16, lw, off, w = c_tiles.pop(m)
                        smt = m_tiles[m]
                        nc.vector.tensor_mul(
                            smt[:, :w], sc16[:, :w], lw[:, off:off + w]
                        )
                if step >= MP and (step - MP) % 4 == 3:
                    # whole-item PV block, tcn-major so each psum
                    # accumulation group closes before the next opens
                    i = (step - MP) // 4
                    _, _, kq, vw, lw = stage_ops(4 * i)
                    diag = i < 2
                    out_ps = psO.tile([128, 256], f32, tag="out")
                    smts = [m_tiles.pop(4 * i + sc) for sc in range(4)]
                    # PV: stationary = Sm^T t-chunk, moving = v chunk ->
                    # out[t, d] accumulated over s-chunks
                    for tcn in range(4):
                        for sc in range(0, (tcn + 1) if diag else 4):
                            col0 = (tcn - sc) * 128 if diag else tcn * 128
                            nc.tensor.matmul(
                                out_ps[:, tcn * 64:(tcn + 1) * 64],
                                smts[sc][:, col0:col0 + 128],
                                vw[:, sc * 64:(sc + 1) * 64],
                                start=(sc == 0),
                                stop=(sc == (tcn if diag else 3)),
                            )
                    if True:
                        st, o0 = (stA, i) if i < 8 else (stB, i - 8)
                        if eng_t["a"] <= eng_t["d"]:
                            eng_t["a"] += 360.0
                            nc.scalar.copy(
                                st[:, o0 * 256:(o0 + 1) * 256], out_ps[:]
                            )
                        else:
                            eng_t["d"] += 400.0
                            nc.vector.tensor_copy(
                                st[:, o0 * 256:(o0 + 1) * 256], out_ps[:]
                            )
                        if i < 8:
                            if i == 5:
                                nc.sync.dma_start(po[:, 0:6 * 256],
                                                  stA[:, 0:6 * 256])
                            if i == 7:
                                nc.sync.dma_start(po[:, 6 * 256:8 * 256],
                                                  stA[:, 6 * 256:])
                        else:
                            j = i - 8
                            if j == 5:
                                nc.sync.dma_start(
                                    po[:, 8 * 256:14 * 256], stB[:, 0:6 * 256]
                                )
                if step < NST and P2_FILL_STEP:
                    filler(P2_FILL_STEP)
            nc.sync.dma_start(po[:, 14 * 256:NITEMS * 256],
                              stB[:, 6 * 256:])
    return nc


_nc_cache = {}


def _get_nc(which):
    if which not in _nc_cache:
        _nc_cache[which] = _build_phase1() if which == 1 else _build_phase2()
    return _nc_cache[which]


# ---------------------------------------------------------------------------
# Host-side packing helpers (pure data movement + dtype cast)
# ---------------------------------------------------------------------------
def _pack_chunks(a, nchunk, rows):
    """[nchunk*rows, w] -> [rows, nchunk*w] with chunk-major free dim."""
    w = a.shape[1]
    return np.ascontiguousarray(
        a.reshape(nchunk, rows, w).transpose(1, 0, 2).reshape(rows, nchunk * w)
    )


def kernel(x, Wq, Wk, Wv, l):
    _install_bir_fix()
    from concourse import bass_utils

    x = np.asarray(x, dtype=np.float32)
    l = np.asarray(l, dtype=np.float32)

    core_ids = list(range(NCORES))

    # ---------------- Phase 1: per-core projections -----------------------
    Wp = np.concatenate(
        [
            _pack_chunks(np.asarray(wm, dtype=np.float16), 4, 128)
            for wm in (Wq, Wk, Wv)
        ],
        axis=1,
    )  # [128, 768]
    in1 = []
    panels = []
    for c in range(NCORES):
        tA = 512 * c
        tB = T - 512 * (c + 1)
        panels.append((tA, tB))
        xcat = np.concatenate([x[tA:tA + 512], x[tB:tB + 512]], axis=0)
        xT = np.ascontiguousarray(xcat.T).astype(np.float16)  # [512, 1024]
        xTp = _pack_chunks(xT, 4, 128)  # [128, 4096]
        in1.append({"xTp": xTp, "Wp": Wp})

    res1 = bass_utils.run_bass_kernel_spmd(_get_nc(1), in1, core_ids=core_ids)

    qT_full = np.empty((64, T), dtype=np.float16)
    kT_full = np.empty((64, T), dtype=np.float16)
    v_full = np.empty((T, 64), dtype=np.float16)
    for c in range(NCORES):
        tA, tB = panels[c]
        r = res1.results[c]
        qk = r["qk_o"]  # [64, 2048]: qA | qB | kA | kB
        qT_full[:, tA:tA + 512] = qk[:, 0:512]
        qT_full[:, tB:tB + 512] = qk[:, 512:1024]
        kT_full[:, tA:tA + 512] = qk[:, 1024:1536]
        kT_full[:, tB:tB + 512] = qk[:, 1536:2048]
        vup = r["v_o"].reshape(128, 8, 64).transpose(1, 0, 2).reshape(1024, 64)
        v_full[tA:tA + 512] = vup[:512]
        v_full[tB:tB + 512] = vup[512:]

    # ---------------- Phase 2: masked scores + PV -------------------------
    in2 = []
    for c in range(NCORES):
        items = _core_items(c)
        lwd = np.empty((128, 2560), dtype=np.float16)
        lwp = np.empty((128, 15 * 2048), dtype=np.float16)
        kqp = np.empty((64, NITEMS * 1024), dtype=np.float16)
        vpk = np.empty((128, NITEMS * 256), dtype=np.float16)
        for i, (t0, s0) in enumerate(items):
            lt = l[t0:t0 + 512, s0:s0 + 512]
            if i < 2:
                lT = np.tril(lt).T.astype(np.float16)  # [512 s, 512 t]
                for sc in range(4):
                    w = 512 - 128 * sc
                    o = i * 1280 + DIAG_OFF[sc]
                    lwd[:, o:o + w] = lT[128 * sc:128 * (sc + 1), 128 * sc:512]
            else:
                lT = lt.T.astype(np.float16)
                lwp[:, (i - 2) * 2048:(i - 1) * 2048] = (
                    lT.reshape(4, 128, 512).transpose(1, 0, 2).reshape(128, 2048)
                )
            kqp[:, i * 1024:i * 1024 + 512] = kT_full[:, s0:s0 + 512]
            kqp[:, i * 1024 + 512:(i + 1) * 1024] = qT_full[:, t0:t0 + 512]
            vpk[:, i * 256:(i + 1) * 256] = (
                v_full[s0:s0 + 512]
                .reshape(4, 128, 64)
                .transpose(1, 0, 2)
                .reshape(128, 256)
            )
        in2.append({"lwd": lwd, "lwp": lwp, "kqp": kqp, "vp": vpk})

    res2 = bass_utils.run_bass_kernel_spmd(_get_nc(2), in2, core_ids=core_ids)

    out = np.empty((T, 64), dtype=np.float32)
    for c in range(NCORES):
        tA, tB = panels[c]
        p = res2.results[c]["po"].astype(np.float32)  # [128, 17*256]
        # unpack per item: [128 t, tc*64 + d] -> [512 t, 64 d]
        pit = (
            p.reshape(128, NITEMS, 4, 64)
            .transpose(1, 2, 0, 3)
            .reshape(NITEMS, 512, 64)
        )
        # item 0 = diag A, item 1 = diag B, 2..2+c full A, rest full B
        out[tA:tA + 512] = pit[0] + pit[2:2 + c].sum(axis=0)
        out[tB:tB + 512] = pit[1] + pit[2 + c:].sum(axis=0)
    return out


# revision 63
# speedup vs baseline: 1.6450x; 1.0302x over previous
"""Trainium2 Bass kernel for masked-attention-like module:
    q = x@Wq; k = x@Wk; v = x@Wv
    scores = (q @ k.T) * tril(l)
    out = scores @ v
T=8192, D_IN=512, D_QK=D_V=64, fp32 inputs/outputs, 8 NeuronCores.

Strategy (sequence-parallel over T, load-balanced over the tril):
  - Work is tiled into [512 t x 512 s] macro-tiles of the lower triangle.
    Core c owns two t-panels: rows [512c, 512c+512) and
    [8192-512(c+1), 8192-512c).  That gives every core exactly 17
    macro-tiles -> identical, branch-free SPMD program.
  - All device operands are fp16, pre-packed on the host (pure data
    movement + dtype cast): l tiles are pre-masked (tril), transposed,
    and cast; x / W are cast for phase 1.
  - Phase 1 (small SPMD kernel): each core computes qT/kT and v (fp16)
    for its own 1024 rows from a host-pre-transposed fp16 x block.
  - Host gathers the tiny projections, then packs per-core per-item
    operands.  Phase 2 per macro-tile, all in transposed score layout:
        S^T[s,t] = sum_n kT[n,s] qT[n,t]      (PE, fp16)
        Sm^T = S^T * lT                       (3-path split, fp16 out)
        out[t,d] += Sm^T[s,t-chunk]^T-free @ v[s,d]  (PE, accumulate)
    The PV matmul uses Sm^T chunks as the stationary operand so the
    output lands in natural [t, d] layout with full 128-partition use
    (half the PE row count of the [d, t] formulation).
  - The score-mask multiplies are statically load-balanced across
    three paths (GPSIMD cannot read PSUM on TRN2): direct DVE fp32,
    ACT-copy + DVE fp16 (2x mode), and ACT-copy + Pool fp16; the last
    items avoid the slow Pool path to shorten the tail.  Scratch
    matmuls prime the PE p-state while the first DMAs are in flight.
  - All input DMAs issue on one queue in exact consumption order (the
    transfer engine is a single serialized resource at 360 GB/s, and
    per-DMA HWDGE setup costs ~625 ns, so both bytes and DMA count are
    minimized via fp16 + batched 2D-packed layouts).
  - Per-item outputs are staged in SBUF and flushed with a few batched
    DMAs; the host sums the per-panel partials (fp32).
"""

import json

import numpy as np

T = 8192
D_IN = 512
D_QK = 64
D_V = 64
NCORES = 8
PANEL = 512  # rows per t-panel
NITEMS = 17  # macro-tiles per core

# ---------------------------------------------------------------------------
# Workaround: the walrus build in this container accepts only ONE sync-wait
# per instruction, but Tile attaches several (e.g. to the tail Drain).  Split
# multi-wait instructions at the BIR-JSON level by inserting single-wait NoOps
# on the same engine immediately before the instruction.
# ---------------------------------------------------------------------------
_fix_installed = [False]
_split_counter = [0]


def _fix_bir_json(bir_json):
    m = json.loads(bir_json)
    for f in m.get("functions", []):
        for blk in f.get("blocks", []):
            new_insts = []
            for inst in blk.get("instructions", []):
                si = inst.get("sync_info") or {}
                waits = si.get("on_wait") or []
                if len(waits) > 1:
                    for w in waits[:-1]:
                        _split_counter[0] += 1
                        new_insts.append({
                            "name": f"I-waitsplit-{_split_counter[0]}",
                            "opcode": "NoOp",
                            "engine": inst.get("engine"),
                            "ins": [],
                            "outs": [],
                            "sync_info": {"on_wait": [w], "on_update": []},
                        })
                    si = dict(si)
                    si["on_wait"] = waits[-1:]
                    inst = dict(inst)
                    inst["sync_info"] = si
                new_insts.append(inst)
            blk["instructions"] = new_insts
    return json.dumps(m).encode()


def _install_bir_fix():
    if _fix_installed[0]:
        return
    _fix_installed[0] = True
    import concourse.bass_utils as bu
    import concourse.bass2jax as b2j

    orig = bu.compile_bir_kernel

    def patched(bir_json, tmpdir, neff_name="file.neff"):
        return orig(_fix_bir_json(bir_json), tmpdir, neff_name)

    bu.compile_bir_kernel = patched
    b2j.compile_bir_kernel = patched


# ---------------------------------------------------------------------------
# Per-core work-item list: (t0, s0) macro-tile origins, 17 per core.
# ---------------------------------------------------------------------------
def _core_items(c):
    """17 macro-tiles: positions 0/1 are the two diagonal tiles (uniform
    across cores), positions 2..16 the fifteen strictly-lower full tiles."""
    tA = 512 * c
    tB = T - 512 * (c + 1)
    items = [(tA, tA), (tB, tB)]
    items += [(tA, 512 * j) for j in range(c)]
    items += [(tB, 512 * j) for j in range(15 - c)]
    assert len(items) == NITEMS
    return items


# kq/vp DMA batching over items: 5 batches.
BATCHES = [(0, 2), (2, 6), (6, 10), (10, 14), (14, 17)]
DIAG_OFF = [0, 512, 896, 1152]  # prefix sums of widths 512,384,256,128

# PE p-state filler tuning (rows of scratch matmul work)
P1_FILL_PRE = 4
P2_FILL_PRE = 6
P2_FILL_STEP = 0
P2_MD = 3  # mult stage trails S stage
P2_SMT_BUFS = 16
P2_MP = 7  # PV stage trails S stage
# lw full-tile DMA batching (indices into the 15 full tiles)
LW_BATCH = [(0, 2), (2, 4), (4, 6), (6, 8), (8, 10), (10, 12), (12, 13),
            (13, 14), (14, 15)]


# ---------------------------------------------------------------------------
# Bass kernel builders
# ---------------------------------------------------------------------------
def _build_phase1():
    import concourse.bass as bass
    import concourse.mybir as mybir
    from concourse.tile import TileContext

    f32 = mybir.dt.float32
    f16 = mybir.dt.float16

    nc = bass.Bass(target_bir_lowering=False, trn_type="TRN2")
    # host-pre-transposed, fp16, d-chunk-major x block: [128 d, dc*1024 + t]
    xTp = nc.dram_tensor("xTp", [128, 4096], f16, kind="ExternalInput")
    # packed weights: wq chunks (4*64) | wk chunks | wv chunks
    Wp = nc.dram_tensor("Wp", [128, 768], f16, kind="ExternalInput")
    qk_o = nc.dram_tensor("qk_o", [64, 2048], f16, kind="ExternalOutput")
    v_o = nc.dram_tensor("v_o", [128, 512], f16, kind="ExternalOutput")

    with TileContext(nc) as tc:
        with (
            tc.tile_pool(name="sb", bufs=1) as sb,
            tc.tile_pool(name="ps", bufs=1, space="PSUM") as ps,
        ):
            w = sb.tile([128, 768], f16, tag="w")
            nc.scalar.dma_start(w[:], Wp[:])
            # d-chunked x DMAs so the PE can start after the first chunk
            xt = {}
            for dc in range(4):
                xc = sb.tile([128, 1024], f16, tag=f"x{dc}", name=f"x{dc}")
                nc.sync.dma_start(xc[:], xTp[:, dc * 1024:(dc + 1) * 1024])
                for h in range(2):
                    xt[(dc, h)] = xc[:, h * 512:(h + 1) * 512]

            # PE p-state priming: the tensor engine clock ramps only under
            # sustained execution, so burn the cold period on scratch
            # matmuls while the first DMAs are in flight; real matmuls then
            # run at full clock.
            scr = sb.tile([128, 512], f16, tag="scr")
            nc.vector.memset(scr[:], 0.0)
            psF = ps.tile([128, 512], f32, tag="psF", name="psF")

            def filler(rows):
                nc.tensor.matmul(
                    psF[:, :rows],
                    scr[:, :128],
                    scr[:, :rows],
                    start=True,
                    stop=True,
                )

            for _ in range(P1_FILL_PRE):
                filler(512)

            # qT/kT: 4 psum groups (q/k x 2 sbuf-bank halves of t),
            # dc-outer order: all groups touch chunk dc before dc+1, so the
            # PE never waits on a chunk that is still in flight.
            pq = [
                ps.tile([64, 512], f32, tag=f"pq{j}", name=f"pq{j}")
                for j in range(4)
            ]
            for dc in range(4):
                for h in range(2):
                    for qk in range(2):
                        j = qk * 2 + h
                        wbase = qk * 256  # 0 -> Wq, 256 -> Wk
                        nc.tensor.matmul(
                            pq[j][:],
                            w[:, wbase + dc * 64:wbase + (dc + 1) * 64],
                            xt[(dc, h)][:],
                            start=(dc == 0),
                            stop=(dc == 3),
                        )
            # copies split between ACT and DVE; each qk half is flushed as
            # soon as its two copies land, on the queue that will not block
            # anything behind it (qk halves on sync, v halves on scalar).
            qk_st = sb.tile([64, 2048], f16, tag="qkst")
            for j in range(4):
                if j % 2 == 0:
                    nc.scalar.copy(qk_st[:, j * 512:(j + 1) * 512], pq[j][:])
                else:
                    nc.vector.tensor_copy(
                        qk_st[:, j * 512:(j + 1) * 512], pq[j][:]
                    )
                if j == 1:
                    nc.sync.dma_start(qk_o[:, 0:1024], qk_st[:, 0:1024])
            nc.sync.dma_start(qk_o[:, 1024:2048], qk_st[:, 1024:2048])

            # v in natural [t, d] layout: stationary x chunks, moving Wv.
            # Two psum banks of 4 t-chunks each: bank B accumulates while
            # bank A is drained in one bulk copy, so the PE never waits.
            v_st = sb.tile([128, 512], f16, tag="vst")
            for half in range(2):
                pvh = ps.tile([128, 256], f32, tag=f"pv{half}",
                              name=f"pv{half}")
                for tq in range(4):
                    tcn = half * 4 + tq
                    for dc in range(4):
                        nc.tensor.matmul(
                            pvh[:, tq * 64:(tq + 1) * 64],
                            xt[(dc, half)][:, tq * 128:(tq + 1) * 128],
                            w[:, 512 + dc * 64:512 + (dc + 1) * 64],
                            start=(dc == 0),
                            stop=(dc == 3),
                        )
                if half == 0:
                    nc.vector.tensor_copy(v_st[:, 0:256], pvh[:])
                    nc.sync.dma_start(v_o[:, 0:256], v_st[:, 0:256])
                else:
                    nc.scalar.copy(v_st[:, 256:512], pvh[:])
            nc.sync.dma_start(v_o[:, 256:512], v_st[:, 256:512])
    return nc


def _build_phase2():
    import concourse.bass as bass
    import concourse.mybir as mybir
    from concourse.tile import TileContext

    f32 = mybir.dt.float32
    f16 = mybir.dt.float16

    nc = bass.Bass(target_bir_lowering=False, trn_type="TRN2")
    # diag tiles, dense-packed lower-tri chunks, both items side by side
    lwd = nc.dram_tensor("lwd", [128, 2560], f16, kind="ExternalInput")
    lwp = nc.dram_tensor("lwp", [128, 15 * 2048], f16, kind="ExternalInput")
    # per item: kT tile (512) | qT tile (512), item-major columns
    kqp = nc.dram_tensor("kqp", [64, NITEMS * 1024], f16, kind="ExternalInput")
    # per item: v tile packed [128 s, sc*64 + d], item-major columns
    vp = nc.dram_tensor("vp", [128, NITEMS * 256], f16, kind="ExternalInput")
    # per item: out packed [128 t, tc*64 + d], item-major columns
    po = nc.dram_tensor("po", [128, NITEMS * 256], f16, kind="ExternalOutput")

    # Static load-balancer for the mask multiplies.  GPSIMD cannot touch
    # PSUM on TRN2, so the three legal paths are:
    #   A: DVE multiplies straight from PSUM (fp32 rate)
    #   B: ACT copies PSUM -> SBUF fp16, DVE multiplies all-fp16 (2x rate)
    #   C: ACT copies PSUM -> SBUF fp16, Pool multiplies (SBUF-only ok)
    eng_t = {"d": 0.0, "a": 0.0, "g": 0.0}

    def pick_path(width, allow_pool=True):
        cA = width * 1.0417 + 125.0
        cBa = width * 0.833 + 230.0
        cBd = width * 0.52 + 60.0
        cCg = width * 1.984 + 30.0
        endA = max(eng_t["d"] + cA, eng_t["a"], eng_t["g"])
        endB = max(eng_t["d"] + cBd, eng_t["a"] + cBa, eng_t["g"])
        endC = max(eng_t["d"], eng_t["a"] + cBa, eng_t["g"] + cCg)
        if not allow_pool:
            endC = float("inf")
        best = min(endA, endB, endC)
        if best == endA:
            eng_t["d"] += cA
            return "A"
        if best == endB:
            eng_t["d"] += cBd
            eng_t["a"] += cBa
            return "B"
        eng_t["a"] += cBa
        eng_t["g"] += cCg
        return "C"

    with TileContext(nc) as tc:
        with (
            tc.tile_pool(name="lw", bufs=1) as lwpool,
            tc.tile_pool(name="ops", bufs=1) as ops,
            tc.tile_pool(name="smt", bufs=8) as smtp,
            tc.tile_pool(name="stage", bufs=1) as stg,
            tc.tile_pool(name="psS", bufs=3, space="PSUM") as psS,
            tc.tile_pool(name="psO", bufs=2, space="PSUM") as psO,
        ):
            # ---- input DMA plan: lw stream on sync, kq/vp stream on scalar
            lwdt = lwpool.tile([128, 2560], f16, tag="lwd")
            kqt, vpt = [], []
            for b, (i0, i1) in enumerate(BATCHES):
                nb = i1 - i0
                kqt.append(
                    ops.tile([64, nb * 1024], f16, tag=f"kq{b}", name=f"kq{b}")
                )
                vpt.append(
                    ops.tile([128, nb * 256], f16, tag=f"vp{b}", name=f"vp{b}")
                )
            lwbt = []
            for j, (j0, j1) in enumerate(LW_BATCH):
                lwbt.append(
                    lwpool.tile(
                        [128, (j1 - j0) * 2048], f16, tag=f"lwb{j}",
                        name=f"lwb{j}",
                    )
                )

            def lw_of(i):
                """SBUF slice holding l tile i (full tiles only, i >= 2)."""
                for j, (j0, j1) in enumerate(LW_BATCH):
                    if j0 <= i - 2 < j1:
                        return lwbt[j][:, (i - 2 - j0) * 2048:(i - 1 - j0) * 2048]
                raise AssertionError

            # Issue ALL input DMAs on the sync queue in exact consumption
            # order: the transfer engine serves one queue in program order,
            # so the kq/vp batches land just-in-time for the S stage without
            # preempting the lw stream that feeds the mask multiplies.
            def kv_batch(b):
                i0, i1 = BATCHES[b]
                nc.sync.dma_start(kqt[b][:], kqp[:, i0 * 1024:i1 * 1024])
                nc.sync.dma_start(vpt[b][:], vp[:, i0 * 256:i1 * 256])

            nc.sync.dma_start(lwdt[:, 0:1280], lwd[:, 0:1280])
            kv_batch(0)
            nc.sync.dma_start(lwdt[:, 1280:2560], lwd[:, 1280:2560])
            kv_batch(1)
            for j, (j0, j1) in enumerate(LW_BATCH):
                if j in (2, 4, 6):
                    kv_batch(2 + (j - 2) // 2)
                nc.sync.dma_start(lwbt[j][:], lwp[:, j0 * 2048:j1 * 2048])

            stA = stg.tile([128, 8 * 256], f16, tag="stA")
            stB = stg.tile([128, 9 * 256], f16, tag="stB")

            # PE p-state priming (see phase 1): scratch matmuls ramp the
            # clock before real work arrives and plug feed gaps after.
            scr = stg.tile([128, 512], f16, tag="scr")
            nc.vector.memset(scr[:], 0.0)
            psF = psO.tile([128, 512], f32, tag="psF", name="psF", bufs=1)

            def filler(rows):
                nc.tensor.matmul(
                    psF[:, :rows],
                    scr[:, :128],
                    scr[:, :rows],
                    start=True,
                    stop=True,
                )

            for _ in range(P2_FILL_PRE):
                filler(512)

            def batch_of(i):
                for b, (i0, i1) in enumerate(BATCHES):
                    if i0 <= i < i1:
                        return b, i - i0
                raise AssertionError

            def geom(i, sc):
                if i < 2:
                    return 512 - 128 * sc, 128 * sc, DIAG_OFF[sc]
                return 512, 0, 512 * sc

            # Software pipeline over 68 (item, sc) stages, three decoupled
            # stage trails: the S matmul for stage s runs at step s, its
            # mask multiply (DVE/Pool) at s+MD, its PV matmuls at s+MP.
            # The deep PV trail keeps the PE queue head from ever waiting
            # on a mult result; scratch fillers plug residual feed gaps so
            # the PE p-state stays at full clock.
            MD = P2_MD
            MP = P2_MP
            NST = NITEMS * 4
            s_tiles = {}
            c_tiles = {}
            m_tiles = {}
            paths = {}
            out_ps = None

            def stage_ops(s):
                i, sc = divmod(s, 4)
                b, bo = batch_of(i)
                return (
                    i,
                    sc,
                    kqt[b][:, bo * 1024:(bo + 1) * 1024],
                    vpt[b][:, bo * 256:(bo + 1) * 256],
                    lwdt[:, i * 1280:(i + 1) * 1280] if i < 2 else lw_of(i),
                )

            for step in range(NST + MP):
                if step < NST:
                    i, sc, kq, vw, lw = stage_ops(step)
                    w, t0, _ = geom(i, sc)
                    s_ps = psS.tile([128, 512], f32, tag="S", bufs=MD + 2)
                    nc.tensor.matmul(
                        s_ps[:, :w],
                        kq[:, sc * 128:(sc + 1) * 128],
                        kq[:, 512 + t0:1024],
                        start=True,
                        stop=True,
                    )
                    s_tiles[step] = s_ps
                if MD <= step < NST + MD:
                    m = step - MD
                    i, sc, kq, vw, lw = stage_ops(m)
                    w, t0, off = geom(i, sc)
                    s_ps = s_tiles.pop(m)
                    smt = smtp.tile([128, 512], f16, tag="smt", bufs=P2_SMT_BUFS)
                    paths[m] = pick_path(w, allow_pool=(m < NST - 8))
                    if paths[m] == "A":
                        nc.vector.tensor_mul(
                            smt[:, :w], s_ps[:, :w], lw[:, off:off + w]
                        )
                    else:
                        sc16 = smtp.tile([128, 512], f16, tag="sc16", bufs=6)
                        nc.scalar.copy(sc16[:, :w], s_ps[:, :w])
                        c_tiles[m] = (sc16, lw, off, w)
                    m_tiles[m] = smt
                if MD + 2 <= step < NST + MD + 2:
                    m = step - MD - 2
                    if paths[m] != "A":
                        sc16, lw, off, w = c_tiles.pop(m)
                        smt = m_tiles[m]
                        eng = nc.vector if paths[m] == "B" else nc.gpsimd
                        eng.tensor_mul(
                            smt[:, :w], sc16[:, :w], lw[:, off:off + w]
                        )
                if step >= MP and (step - MP) % 4 == 3:
                    # whole-item PV block, tcn-major so each psum
                    # accumulation group closes before the next opens
                    i = (step - MP) // 4
                    _, _, kq, vw, lw = stage_ops(4 * i)
                    diag = i < 2
                    out_ps = psO.tile([128, 256], f32, tag="out")
                    smts = [m_tiles.pop(4 * i + sc) for sc in range(4)]
                    # PV: stationary = Sm^T t-chunk, moving = v chunk ->
                    # out[t, d] accumulated over s-chunks
                    for tcn in range(4):
                        for sc in range(0, (tcn + 1) if diag else 4):
                            col0 = (tcn - sc) * 128 if diag else tcn * 128
                            nc.tensor.matmul(
                                out_ps[:, tcn * 64:(tcn + 1) * 64],
                                smts[sc][:, col0:col0 + 128],
                                vw[:, sc * 64:(sc + 1) * 64],
                                start=(sc == 0),
                                stop=(sc == (tcn if diag else 3)),
                            )
                    if True:
                        st, o0 = (stA, i) if i < 8 else (stB, i - 8)
                        if eng_t["a"] <= eng_t["d"]:
                            eng_t["a"] += 360.0
                            nc.scalar.copy(
                                st[:, o0 * 256:(o0 + 1) * 256], out_ps[:]
                            )
                        else:
                            eng_t["d"] += 400.0
                            nc.vector.tensor_copy(
                                st[:, o0 * 256:(o0 + 1) * 256], out_ps[:]
                            )
                        if i < 8:
                            if i == 5:
                                nc.sync.dma_start(po[:, 0:6 * 256],
                                                  stA[:, 0:6 * 256])
                            if i == 7:
                                nc.sync.dma_start(po[:, 6 * 256:8 * 256],
                                                  stA[:, 6 * 256:])
                        else:
                            j = i - 8
                            if j == 5:
                                nc.sync.dma_start(
                                    po[:, 8 * 256:14 * 256], stB[:, 0:6 * 256]
                                )
                            if j == 7:
                                nc.sync.dma_start(
                                    po[:, 14 * 256:16 * 256],
                                    stB[:, 6 * 256:8 * 256],
                                )
                if step < NST and P2_FILL_STEP:
                    filler(P2_FILL_STEP)
            nc.sync.dma_start(po[:, 16 * 256:NITEMS * 256],
                              stB[:, 8 * 256:])
    return nc


_nc_cache = {}


def _get_nc(which):
    if which not in _nc_cache:
        _nc_cache[which] = _build_phase1() if which == 1 else _build_phase2()
    return _nc_cache[which]


# ---------------------------------------------------------------------------
# Host-side packing helpers (pure data movement + dtype cast)
# ---------------------------------------------------------------------------
def _pack_chunks(a, nchunk, rows):
    """[nchunk*rows, w] -> [rows, nchunk*w] with chunk-major free dim."""
    w = a.shape[1]
    return np.ascontiguousarray(
        a.reshape(nchunk, rows, w).transpose(1, 0, 2).reshape(rows, nchunk * w)
    )


def kernel(x, Wq, Wk, Wv, l):
    _install_bir_fix()
    from concourse import bass_utils

    x = np.asarray(x, dtype=np.float32)
    l = np.asarray(l, dtype=np.float32)

    core_ids = list(range(NCORES))

    # ---------------- Phase 1: per-core projections -----------------------
    Wp = np.concatenate(
        [
            _pack_chunks(np.asarray(wm, dtype=np.float16), 4, 128)
            for wm in (Wq, Wk, Wv)
        ],
        axis=1,
    )  # [128, 768]
    in1 = []
    panels = []
    for c in range(NCORES):
        tA = 512 * c
        tB = T - 512 * (c + 1)
        panels.append((tA, tB))
        xcat = np.concatenate([x[tA:tA + 512], x[tB:tB + 512]], axis=0)
        xT = np.ascontiguousarray(xcat.T).astype(np.float16)  # [512, 1024]
        xTp = _pack_chunks(xT, 4, 128)  # [128, 4096]
        in1.append({"xTp": xTp, "Wp": Wp})

    res1 = bass_utils.run_bass_kernel_spmd(_get_nc(1), in1, core_ids=core_ids)

    qT_full = np.empty((64, T), dtype=np.float16)
    kT_full = np.empty((64, T), dtype=np.float16)
    v_full = np.empty((T, 64), dtype=np.float16)
    for c in range(NCORES):
        tA, tB = panels[c]
        r = res1.results[c]
        qk = r["qk_o"]  # [64, 2048]: qA | qB | kA | kB
        qT_full[:, tA:tA + 512] = qk[:, 0:512]
        qT_full[:, tB:tB + 512] = qk[:, 512:1024]
        kT_full[:, tA:tA + 512] = qk[:, 1024:1536]
        kT_full[:, tB:tB + 512] = qk[:, 1536:2048]
        vup = r["v_o"].reshape(128, 8, 64).transpose(1, 0, 2).reshape(1024, 64)
        v_full[tA:tA + 512] = vup[:512]
        v_full[tB:tB + 512] = vup[512:]

    # ---------------- Phase 2: masked scores + PV -------------------------
    in2 = []
    for c in range(NCORES):
        items = _core_items(c)
        lwd = np.empty((128, 2560), dtype=np.float16)
        lwp = np.empty((128, 15 * 2048), dtype=np.float16)
        kqp = np.empty((64, NITEMS * 1024), dtype=np.float16)
        vpk = np.empty((128, NITEMS * 256), dtype=np.float16)
        for i, (t0, s0) in enumerate(items):
            lt = l[t0:t0 + 512, s0:s0 + 512]
            if i < 2:
                lT = np.tril(lt).T.astype(np.float16)  # [512 s, 512 t]
                for sc in range(4):
                    w = 512 - 128 * sc
                    o = i * 1280 + DIAG_OFF[sc]
                    lwd[:, o:o + w] = lT[128 * sc:128 * (sc + 1), 128 * sc:512]
            else:
                lT = lt.T.astype(np.float16)
                lwp[:, (i - 2) * 2048:(i - 1) * 2048] = (
                    lT.reshape(4, 128, 512).transpose(1, 0, 2).reshape(128, 2048)
                )
            kqp[:, i * 1024:i * 1024 + 512] = kT_full[:, s0:s0 + 512]
            kqp[:, i * 1024 + 512:(i + 1) * 1024] = qT_full[:, t0:t0 + 512]
            vpk[:, i * 256:(i + 1) * 256] = (
                v_full[s0:s0 + 512]
                .reshape(4, 128, 64)
                .transpose(1, 0, 2)
                .reshape(128, 256)
            )
        in2.append({"lwd": lwd, "lwp": lwp, "kqp": kqp, "vp": vpk})

    res2 = bass_utils.run_bass_kernel_spmd(_get_nc(2), in2, core_ids=core_ids)

    out = np.empty((T, 64), dtype=np.float32)
    for c in range(NCORES):
        tA, tB = panels[c]
        p = res2.results[c]["po"].astype(np.float32)  # [128, 17*256]
        # unpack per item: [128 t, tc*64 + d] -> [512 t, 64 d]
        pit = (
            p.reshape(128, NITEMS, 4, 64)
            .transpose(1, 2, 0, 3)
            .reshape(NITEMS, 512, 64)
        )
        # item 0 = diag A, item 1 = diag B, 2..2+c full A, rest full B
        out[tA:tA + 512] = pit[0] + pit[2:2 + c].sum(axis=0)
        out[tB:tB + 512] = pit[1] + pit[2 + c:].sum(axis=0)
    return out


# revision 65
# speedup vs baseline: 1.6513x; 1.0038x over previous
"""Trainium2 Bass kernel for masked-attention-like module:
    q = x@Wq; k = x@Wk; v = x@Wv
    scores = (q @ k.T) * tril(l)
    out = scores @ v
T=8192, D_IN=512, D_QK=D_V=64, fp32 inputs/outputs, 8 NeuronCores.

Strategy (sequence-parallel over T, load-balanced over the tril):
  - Work is tiled into [512 t x 512 s] macro-tiles of the lower triangle.
    Core c owns two t-panels: rows [512c, 512c+512) and
    [8192-512(c+1), 8192-512c).  That gives every core exactly 17
    macro-tiles -> identical, branch-free SPMD program.
  - All device operands are fp16, pre-packed on the host (pure data
    movement + dtype cast): l tiles are pre-masked (tril), transposed,
    and cast; x / W are cast for phase 1.
  - Phase 1 (small SPMD kernel): each core computes qT/kT and v (fp16)
    for its own 1024 rows from a host-pre-transposed fp16 x block.
  - Host gathers the tiny projections, then packs per-core per-item
    operands.  Phase 2 per macro-tile, all in transposed score layout:
        S^T[s,t] = sum_n kT[n,s] qT[n,t]      (PE, fp16)
        Sm^T = S^T * lT                       (3-path split, fp16 out)
        out[t,d] += Sm^T[s,t-chunk]^T-free @ v[s,d]  (PE, accumulate)
    The PV matmul uses Sm^T chunks as the stationary operand so the
    output lands in natural [t, d] layout with full 128-partition use
    (half the PE row count of the [d, t] formulation).
  - The score-mask multiplies are statically load-balanced across
    three paths (GPSIMD cannot read PSUM on TRN2): direct DVE fp32,
    ACT-copy + DVE fp16 (2x mode), and ACT-copy + Pool fp16; the last
    items avoid the slow Pool path to shorten the tail.  Scratch
    matmuls prime the PE p-state while the first DMAs are in flight.
  - All input DMAs issue on one queue in exact consumption order (the
    transfer engine is a single serialized resource at 360 GB/s, and
    per-DMA HWDGE setup costs ~625 ns, so both bytes and DMA count are
    minimized via fp16 + batched 2D-packed layouts).
  - Per-item outputs are staged in SBUF and flushed with a few batched
    DMAs; the host sums the per-panel partials (fp32).
"""

import json

import numpy as np

T = 8192
D_IN = 512
D_QK = 64
D_V = 64
NCORES = 8
PANEL = 512  # rows per t-panel
NITEMS = 17  # macro-tiles per core

# ---------------------------------------------------------------------------
# Workaround: the walrus build in this container accepts only ONE sync-wait
# per instruction, but Tile attaches several (e.g. to the tail Drain).  Split
# multi-wait instructions at the BIR-JSON level by inserting single-wait NoOps
# on the same engine immediately before the instruction.
# ---------------------------------------------------------------------------
_fix_installed = [False]
_split_counter = [0]


def _fix_bir_json(bir_json):
    m = json.loads(bir_json)
    for f in m.get("functions", []):
        for blk in f.get("blocks", []):
            new_insts = []
            for inst in blk.get("instructions", []):
                si = inst.get("sync_info") or {}
                waits = si.get("on_wait") or []
                if len(waits) > 1:
                    for w in waits[:-1]:
                        _split_counter[0] += 1
                        new_insts.append({
                            "name": f"I-waitsplit-{_split_counter[0]}",
                            "opcode": "NoOp",
                            "engine": inst.get("engine"),
                            "ins": [],
                            "outs": [],
                            "sync_info": {"on_wait": [w], "on_update": []},
                        })
                    si = dict(si)
                    si["on_wait"] = waits[-1:]
                    inst = dict(inst)
                    inst["sync_info"] = si
                new_insts.append(inst)
            blk["instructions"] = new_insts
    return json.dumps(m).encode()


def _install_bir_fix():
    if _fix_installed[0]:
        return
    _fix_installed[0] = True
    import concourse.bass_utils as bu
    import concourse.bass2jax as b2j

    orig = bu.compile_bir_kernel

    def patched(bir_json, tmpdir, neff_name="file.neff"):
        return orig(_fix_bir_json(bir_json), tmpdir, neff_name)

    bu.compile_bir_kernel = patched
    b2j.compile_bir_kernel = patched


# ---------------------------------------------------------------------------
# Per-core work-item list: (t0, s0) macro-tile origins, 17 per core.
# ---------------------------------------------------------------------------
def _core_items(c):
    """17 macro-tiles: positions 0/1 are the two diagonal tiles (uniform
    across cores), positions 2..16 the fifteen strictly-lower full tiles."""
    tA = 512 * c
    tB = T - 512 * (c + 1)
    items = [(tA, tA), (tB, tB)]
    items += [(tA, 512 * j) for j in range(c)]
    items += [(tB, 512 * j) for j in range(15 - c)]
    assert len(items) == NITEMS
    return items


# kq/vp DMA batching over items: 5 batches.
BATCHES = [(0, 2), (2, 6), (6, 10), (10, 14), (14, 17)]
DIAG_OFF = [0, 512, 896, 1152]  # prefix sums of widths 512,384,256,128

# PE p-state filler tuning (rows of scratch matmul work)
P1_FILL_PRE = 4
P2_FILL_PRE = 6
P2_FILL_STEP = 0
P2_MD = 3  # mult stage trails S stage
P2_BD = 1  # path-B second stage trails its ACT copy
P2_SMT_BUFS = 16
P2_MP = 7  # PV stage trails S stage
# lw full-tile DMA batching (indices into the 15 full tiles)
LW_BATCH = [(0, 2), (2, 4), (4, 6), (6, 8), (8, 10), (10, 12), (12, 13),
            (13, 14), (14, 15)]


# ---------------------------------------------------------------------------
# Bass kernel builders
# ---------------------------------------------------------------------------
def _build_phase1():
    import concourse.bass as bass
    import concourse.mybir as mybir
    from concourse.tile import TileContext

    f32 = mybir.dt.float32
    f16 = mybir.dt.float16

    nc = bass.Bass(target_bir_lowering=False, trn_type="TRN2")
    # host-pre-transposed, fp16, d-chunk-major x block: [128 d, dc*1024 + t]
    xTp = nc.dram_tensor("xTp", [128, 4096], f16, kind="ExternalInput")
    # packed weights: wq chunks (4*64) | wk chunks | wv chunks
    Wp = nc.dram_tensor("Wp", [128, 768], f16, kind="ExternalInput")
    qk_o = nc.dram_tensor("qk_o", [64, 2048], f16, kind="ExternalOutput")
    v_o = nc.dram_tensor("v_o", [128, 512], f16, kind="ExternalOutput")

    with TileContext(nc) as tc:
        with (
            tc.tile_pool(name="sb", bufs=1) as sb,
            tc.tile_pool(name="ps", bufs=1, space="PSUM") as ps,
        ):
            w = sb.tile([128, 768], f16, tag="w")
            nc.scalar.dma_start(w[:], Wp[:])
            # d-chunked x DMAs so the PE can start after the first chunk
            xt = {}
            for dc in range(4):
                xc = sb.tile([128, 1024], f16, tag=f"x{dc}", name=f"x{dc}")
                nc.sync.dma_start(xc[:], xTp[:, dc * 1024:(dc + 1) * 1024])
                for h in range(2):
                    xt[(dc, h)] = xc[:, h * 512:(h + 1) * 512]

            # PE p-state priming: the tensor engine clock ramps only under
            # sustained execution, so burn the cold period on scratch
            # matmuls while the first DMAs are in flight; real matmuls then
            # run at full clock.
            scr = sb.tile([128, 512], f16, tag="scr")
            nc.vector.memset(scr[:], 0.0)
            psF = ps.tile([128, 512], f32, tag="psF", name="psF")

            def filler(rows):
                nc.tensor.matmul(
                    psF[:, :rows],
                    scr[:, :128],
                    scr[:, :rows],
                    start=True,
                    stop=True,
                )

            for _ in range(P1_FILL_PRE):
                filler(512)

            # qT/kT: 4 psum groups (q/k x 2 sbuf-bank halves of t),
            # dc-outer order: all groups touch chunk dc before dc+1, so the
            # PE never waits on a chunk that is still in flight.
            pq = [
                ps.tile([64, 512], f32, tag=f"pq{j}", name=f"pq{j}")
                for j in range(4)
            ]
            for dc in range(4):
                for h in range(2):
                    for qk in range(2):
                        j = qk * 2 + h
                        wbase = qk * 256  # 0 -> Wq, 256 -> Wk
                        nc.tensor.matmul(
                            pq[j][:],
                            w[:, wbase + dc * 64:wbase + (dc + 1) * 64],
                            xt[(dc, h)][:],
                            start=(dc == 0),
                            stop=(dc == 3),
                        )
            # copies split between ACT and DVE; each qk half is flushed as
            # soon as its two copies land, on the queue that will not block
            # anything behind it (qk halves on sync, v halves on scalar).
            qk_st = sb.tile([64, 2048], f16, tag="qkst")
            for j in range(4):
                if j % 2 == 0:
                    nc.scalar.copy(qk_st[:, j * 512:(j + 1) * 512], pq[j][:])
                else:
                    nc.vector.tensor_copy(
                        qk_st[:, j * 512:(j + 1) * 512], pq[j][:]
                    )
                if j == 1:
                    nc.sync.dma_start(qk_o[:, 0:1024], qk_st[:, 0:1024])
            nc.sync.dma_start(qk_o[:, 1024:2048], qk_st[:, 1024:2048])

            # v in natural [t, d] layout: stationary x chunks, moving Wv.
            # Two psum banks of 4 t-chunks each: bank B accumulates while
            # bank A is drained in one bulk copy, so the PE never waits.
            v_st = sb.tile([128, 512], f16, tag="vst")
            for half in range(2):
                pvh = ps.tile([128, 256], f32, tag=f"pv{half}",
                              name=f"pv{half}")
                for tq in range(4):
                    tcn = half * 4 + tq
                    for dc in range(4):
                        nc.tensor.matmul(
                            pvh[:, tq * 64:(tq + 1) * 64],
                            xt[(dc, half)][:, tq * 128:(tq + 1) * 128],
                            w[:, 512 + dc * 64:512 + (dc + 1) * 64],
                            start=(dc == 0),
                            stop=(dc == 3),
                        )
                if half == 0:
                    nc.vector.tensor_copy(v_st[:, 0:256], pvh[:])
                    nc.sync.dma_start(v_o[:, 0:256], v_st[:, 0:256])
                else:
                    nc.scalar.copy(v_st[:, 256:512], pvh[:])
            nc.sync.dma_start(v_o[:, 256:512], v_st[:, 256:512])
    return nc


def _build_phase2():
    import concourse.bass as bass
    import concourse.mybir as mybir
    from concourse.tile import TileContext

    f32 = mybir.dt.float32
    f16 = mybir.dt.float16

    nc = bass.Bass(target_bir_lowering=False, trn_type="TRN2")
    # diag tiles, dense-packed lower-tri chunks, both items side by side
    lwd = nc.dram_tensor("lwd", [128, 2560], f16, kind="ExternalInput")
    lwp = nc.dram_tensor("lwp", [128, 15 * 2048], f16, kind="ExternalInput")
    # per item: kT tile (512) | qT tile (512), item-major columns
    kqp = nc.dram_tensor("kqp", [64, NITEMS * 1024], f16, kind="ExternalInput")
    # per item: v tile packed [128 s, sc*64 + d], item-major columns
    vp = nc.dram_tensor("vp", [128, NITEMS * 256], f16, kind="ExternalInput")
    # per item: out packed [128 t, tc*64 + d], item-major columns
    po = nc.dram_tensor("po", [128, NITEMS * 256], f16, kind="ExternalOutput")

    # Static load-balancer for the mask multiplies.  GPSIMD cannot touch
    # PSUM on TRN2, so the three legal paths are:
    #   A: DVE multiplies straight from PSUM (fp32 rate)
    #   B: ACT copies PSUM -> SBUF fp16, DVE multiplies all-fp16 (2x rate)
    #   C: ACT copies PSUM -> SBUF fp16, Pool multiplies (SBUF-only ok)
    eng_t = {"d": 0.0, "a": 0.0, "g": 0.0}

    def pick_path(width, allow_pool=True):
        cA = width * 1.0417 + 125.0
        cBa = width * 0.833 + 230.0
        cBd = width * 0.52 + 60.0
        cCg = width * 1.984 + 30.0
        endA = max(eng_t["d"] + cA, eng_t["a"], eng_t["g"])
        endB = max(eng_t["d"] + cBd, eng_t["a"] + cBa, eng_t["g"])
        endC = max(eng_t["d"], eng_t["a"] + cBa, eng_t["g"] + cCg)
        if not allow_pool:
            endC = float("inf")
        best = min(endA, endB, endC)
        if best == endA:
            eng_t["d"] += cA
            return "A"
        if best == endB:
            eng_t["d"] += cBd
            eng_t["a"] += cBa
            return "B"
        eng_t["a"] += cBa
        eng_t["g"] += cCg
        return "C"

    with TileContext(nc) as tc:
        with (
            tc.tile_pool(name="lw", bufs=1) as lwpool,
            tc.tile_pool(name="ops", bufs=1) as ops,
            tc.tile_pool(name="smt", bufs=8) as smtp,
            tc.tile_pool(name="stage", bufs=1) as stg,
            tc.tile_pool(name="psS", bufs=3, space="PSUM") as psS,
            tc.tile_pool(name="psO", bufs=2, space="PSUM") as psO,
        ):
            # ---- input DMA plan: lw stream on sync, kq/vp stream on scalar
            lwdt = lwpool.tile([128, 2560], f16, tag="lwd")
            kqt, vpt = [], []
            for b, (i0, i1) in enumerate(BATCHES):
                nb = i1 - i0
                kqt.append(
                    ops.tile([64, nb * 1024], f16, tag=f"kq{b}", name=f"kq{b}")
                )
                vpt.append(
                    ops.tile([128, nb * 256], f16, tag=f"vp{b}", name=f"vp{b}")
                )
            lwbt = []
            for j, (j0, j1) in enumerate(LW_BATCH):
                lwbt.append(
                    lwpool.tile(
                        [128, (j1 - j0) * 2048], f16, tag=f"lwb{j}",
                        name=f"lwb{j}",
                    )
                )

            def lw_of(i):
                """SBUF slice holding l tile i (full tiles only, i >= 2)."""
                for j, (j0, j1) in enumerate(LW_BATCH):
                    if j0 <= i - 2 < j1:
                        return lwbt[j][:, (i - 2 - j0) * 2048:(i - 1 - j0) * 2048]
                raise AssertionError

            # Issue ALL input DMAs on the sync queue in exact consumption
            # order: the transfer engine serves one queue in program order,
            # so the kq/vp batches land just-in-time for the S stage without
            # preempting the lw stream that feeds the mask multiplies.
            def kv_batch(b):
                i0, i1 = BATCHES[b]
                nc.sync.dma_start(kqt[b][:], kqp[:, i0 * 1024:i1 * 1024])
                nc.sync.dma_start(vpt[b][:], vp[:, i0 * 256:i1 * 256])

            nc.sync.dma_start(lwdt[:, 0:1280], lwd[:, 0:1280])
            kv_batch(0)
            nc.sync.dma_start(lwdt[:, 1280:2560], lwd[:, 1280:2560])
            kv_batch(1)
            for j, (j0, j1) in enumerate(LW_BATCH):
                if j in (2, 4, 6):
                    kv_batch(2 + (j - 2) // 2)
                nc.sync.dma_start(lwbt[j][:], lwp[:, j0 * 2048:j1 * 2048])

            stA = stg.tile([128, 8 * 256], f16, tag="stA")
            stB = stg.tile([128, 9 * 256], f16, tag="stB")

            # PE p-state priming (see phase 1): scratch matmuls ramp the
            # clock before real work arrives and plug feed gaps after.
            scr = stg.tile([128, 512], f16, tag="scr")
            nc.vector.memset(scr[:], 0.0)
            psF = psO.tile([128, 512], f32, tag="psF", name="psF", bufs=1)

            def filler(rows):
                nc.tensor.matmul(
                    psF[:, :rows],
                    scr[:, :128],
                    scr[:, :rows],
                    start=True,
                    stop=True,
                )

            for _ in range(P2_FILL_PRE):
                filler(512)

            def batch_of(i):
                for b, (i0, i1) in enumerate(BATCHES):
                    if i0 <= i < i1:
                        return b, i - i0
                raise AssertionError

            def geom(i, sc):
                if i < 2:
                    return 512 - 128 * sc, 128 * sc, DIAG_OFF[sc]
                return 512, 0, 512 * sc

            # Software pipeline over 68 (item, sc) stages, three decoupled
            # stage trails: the S matmul for stage s runs at step s, its
            # mask multiply (DVE/Pool) at s+MD, its PV matmuls at s+MP.
            # The deep PV trail keeps the PE queue head from ever waiting
            # on a mult result; scratch fillers plug residual feed gaps so
            # the PE p-state stays at full clock.
            MD = P2_MD
            MP = P2_MP
            NST = NITEMS * 4
            s_tiles = {}
            c_tiles = {}
            m_tiles = {}
            paths = {}
            out_ps = None

            def stage_ops(s):
                i, sc = divmod(s, 4)
                b, bo = batch_of(i)
                return (
                    i,
                    sc,
                    kqt[b][:, bo * 1024:(bo + 1) * 1024],
                    vpt[b][:, bo * 256:(bo + 1) * 256],
                    lwdt[:, i * 1280:(i + 1) * 1280] if i < 2 else lw_of(i),
                )

            for step in range(NST + MP):
                if step < NST:
                    i, sc, kq, vw, lw = stage_ops(step)
                    w, t0, _ = geom(i, sc)
                    s_ps = psS.tile([128, 512], f32, tag="S", bufs=MD + 2)
                    nc.tensor.matmul(
                        s_ps[:, :w],
                        kq[:, sc * 128:(sc + 1) * 128],
                        kq[:, 512 + t0:1024],
                        start=True,
                        stop=True,
                    )
                    s_tiles[step] = s_ps
                if MD <= step < NST + MD:
                    m = step - MD
                    i, sc, kq, vw, lw = stage_ops(m)
                    w, t0, off = geom(i, sc)
                    s_ps = s_tiles.pop(m)
                    smt = smtp.tile([128, 512], f16, tag="smt", bufs=P2_SMT_BUFS)
                    paths[m] = pick_path(w, allow_pool=(m < NST - 8))
                    if paths[m] == "A":
                        nc.vector.tensor_mul(
                            smt[:, :w], s_ps[:, :w], lw[:, off:off + w]
                        )
                    else:
                        sc16 = smtp.tile([128, 512], f16, tag="sc16", bufs=6)
                        nc.scalar.copy(sc16[:, :w], s_ps[:, :w])
                        c_tiles[m] = (sc16, lw, off, w)
                    m_tiles[m] = smt
                if MD + P2_BD <= step < NST + MD + P2_BD:
                    m = step - MD - P2_BD
                    if paths[m] != "A":
                        sc16, lw, off, w = c_tiles.pop(m)
                        smt = m_tiles[m]
                        eng = nc.vector if paths[m] == "B" else nc.gpsimd
                        eng.tensor_mul(
                            smt[:, :w], sc16[:, :w], lw[:, off:off + w]
                        )
                if step >= MP and (step - MP) % 4 == 3:
                    # whole-item PV block, tcn-major so each psum
                    # accumulation group closes before the next opens
                    i = (step - MP) // 4
                    _, _, kq, vw, lw = stage_ops(4 * i)
                    diag = i < 2
                    out_ps = psO.tile([128, 256], f32, tag="out")
                    smts = [m_tiles.pop(4 * i + sc) for sc in range(4)]
                    # PV: stationary = Sm^T t-chunk, moving = v chunk ->
                    # out[t, d] accumulated over s-chunks
                    for tcn in range(4):
                        for sc in range(0, (tcn + 1) if diag else 4):
                            col0 = (tcn - sc) * 128 if diag else tcn * 128
                            nc.tensor.matmul(
                                out_ps[:, tcn * 64:(tcn + 1) * 64],
                                smts[sc][:, col0:col0 + 128],
                                vw[:, sc * 64:(sc + 1) * 64],
                                start=(sc == 0),
                                stop=(sc == (tcn if diag else 3)),
                            )
                    if True:
                        st, o0 = (stA, i) if i < 8 else (stB, i - 8)
                        if eng_t["a"] <= eng_t["d"]:
                            eng_t["a"] += 360.0
                            nc.scalar.copy(
                                st[:, o0 * 256:(o0 + 1) * 256], out_ps[:]
                            )
                        else:
                            eng_t["d"] += 400.0
                            nc.vector.tensor_copy(
                                st[:, o0 * 256:(o0 + 1) * 256], out_ps[:]
                            )
                        if i < 8:
                            if i == 5:
                                nc.sync.dma_start(po[:, 0:6 * 256],
                                                  stA[:, 0:6 * 256])
                            if i == 7:
                                nc.sync.dma_start(po[:, 6 * 256:8 * 256],
                                                  stA[:, 6 * 256:])
                        else:
                            j = i - 8
                            if j == 5:
                                nc.sync.dma_start(
                                    po[:, 8 * 256:14 * 256], stB[:, 0:6 * 256]
                                )
                            if j == 7:
                                nc.sync.dma_start(
                                    po[:, 14 * 256:16 * 256],
                                    stB[:, 6 * 256:8 * 256],
                                )
                if step < NST and P2_FILL_STEP:
                    filler(P2_FILL_STEP)
            nc.sync.dma_start(po[:, 16 * 256:NITEMS * 256],
                              stB[:, 8 * 256:])
    return nc


_nc_cache = {}


def _get_nc(which):
    if which not in _nc_cache:
        _nc_cache[which] = _build_phase1() if which == 1 else _build_phase2()
    return _nc_cache[which]


# ---------------------------------------------------------------------------
# Host-side packing helpers (pure data movement + dtype cast)
# ---------------------------------------------------------------------------
def _pack_chunks(a, nchunk, rows):
    """[nchunk*rows, w] -> [rows, nchunk*w] with chunk-major free dim."""
    w = a.shape[1]
    return np.ascontiguousarray(
        a.reshape(nchunk, rows, w).transpose(1, 0, 2).reshape(rows, nchunk * w)
    )


def kernel(x, Wq, Wk, Wv, l):
    _install_bir_fix()
    from concourse import bass_utils

    x = np.asarray(x, dtype=np.float32)
    l = np.asarray(l, dtype=np.float32)

    core_ids = list(range(NCORES))

    # ---------------- Phase 1: per-core projections -----------------------
    Wp = np.concatenate(
        [
            _pack_chunks(np.asarray(wm, dtype=np.float16), 4, 128)
            for wm in (Wq, Wk, Wv)
        ],
        axis=1,
    )  # [128, 768]
    in1 = []
    panels = []
    for c in range(NCORES):
        tA = 512 * c
        tB = T - 512 * (c + 1)
        panels.append((tA, tB))
        xcat = np.concatenate([x[tA:tA + 512], x[tB:tB + 512]], axis=0)
        xT = np.ascontiguousarray(xcat.T).astype(np.float16)  # [512, 1024]
        xTp = _pack_chunks(xT, 4, 128)  # [128, 4096]
        in1.append({"xTp": xTp, "Wp": Wp})

    res1 = bass_utils.run_bass_kernel_spmd(_get_nc(1), in1, core_ids=core_ids)

    qT_full = np.empty((64, T), dtype=np.float16)
    kT_full = np.empty((64, T), dtype=np.float16)
    v_full = np.empty((T, 64), dtype=np.float16)
    for c in range(NCORES):
        tA, tB = panels[c]
        r = res1.results[c]
        qk = r["qk_o"]  # [64, 2048]: qA | qB | kA | kB
        qT_full[:, tA:tA + 512] = qk[:, 0:512]
        qT_full[:, tB:tB + 512] = qk[:, 512:1024]
        kT_full[:, tA:tA + 512] = qk[:, 1024:1536]
        kT_full[:, tB:tB + 512] = qk[:, 1536:2048]
        vup = r["v_o"].reshape(128, 8, 64).transpose(1, 0, 2).reshape(1024, 64)
        v_full[tA:tA + 512] = vup[:512]
        v_full[tB:tB + 512] = vup[512:]

    # ---------------- Phase 2: masked scores + PV -------------------------
    in2 = []
    for c in range(NCORES):
        items = _core_items(c)
        lwd = np.empty((128, 2560), dtype=np.float16)
        lwp = np.empty((128, 15 * 2048), dtype=np.float16)
        kqp = np.empty((64, NITEMS * 1024), dtype=np.float16)
        vpk = np.empty((128, NITEMS * 256), dtype=np.float16)
        for i, (t0, s0) in enumerate(items):
            lt = l[t0:t0 + 512, s0:s0 + 512]
            if i < 2:
                lT = np.tril(lt).T.astype(np.float16)  # [512 s, 512 t]
                for sc in range(4):
                    w = 512 - 128 * sc
                    o = i * 1280 + DIAG_OFF[sc]
                    lwd[:, o:o + w] = lT[128 * sc:128 * (sc + 1), 128 * sc:512]
            else:
                lT = lt.T.astype(np.float16)
                lwp[:, (i - 2) * 2048:(i - 1) * 2048] = (
                    lT.reshape(4, 128, 512).transpose(1, 0, 2).reshape(128, 2048)
                )
            kqp[:, i * 1024:i * 1024 + 512] = kT_full[:, s0:s0 + 512]
            kqp[:, i * 1024 + 512:(i + 1) * 1024] = qT_full[:, t0:t0 + 512]
            vpk[:, i * 256:(i + 1) * 256] = (
                v_full[s0:s0 + 512]
                .reshape(4, 128, 64)
                .transpose(1, 0, 2)
                .reshape(128, 256)
            )
        in2.append({"lwd": lwd, "lwp": lwp, "kqp": kqp, "vp": vpk})

    res2 = bass_utils.run_bass_kernel_spmd(_get_nc(2), in2, core_ids=core_ids)

    out = np.empty((T, 64), dtype=np.float32)
    for c in range(NCORES):
        tA, tB = panels[c]
        p = res2.results[c]["po"].astype(np.float32)  # [128, 17*256]
        # unpack per item: [128 t, tc*64 + d] -> [512 t, 64 d]
        pit = (
            p.reshape(128, NITEMS, 4, 64)
            .transpose(1, 2, 0, 3)
            .reshape(NITEMS, 512, 64)
        )
        # item 0 = diag A, item 1 = diag B, 2..2+c full A, rest full B
        out[tA:tA + 512] = pit[0] + pit[2:2 + c].sum(axis=0)
        out[tB:tB + 512] = pit[1] + pit[2 + c:].sum(axis=0)
    return out


# revision 66
# speedup vs baseline: 1.6709x; 1.0119x over previous
"""Trainium2 Bass kernel for masked-attention-like module:
    q = x@Wq; k = x@Wk; v = x@Wv
    scores = (q @ k.T) * tril(l)
    out = scores @ v
T=8192, D_IN=512, D_QK=D_V=64, fp32 inputs/outputs, 8 NeuronCores.

Strategy (sequence-parallel over T, load-balanced over the tril):
  - Work is tiled into [512 t x 512 s] macro-tiles of the lower triangle.
    Core c owns two t-panels: rows [512c, 512c+512) and
    [8192-512(c+1), 8192-512c).  That gives every core exactly 17
    macro-tiles -> identical, branch-free SPMD program.
  - All device operands are fp16, pre-packed on the host (pure data
    movement + dtype cast): l tiles are pre-masked (tril), transposed,
    and cast; x / W are cast for phase 1.
  - Phase 1 (small SPMD kernel): each core computes qT/kT and v (fp16)
    for its own 1024 rows from a host-pre-transposed fp16 x block.
  - Host gathers the tiny projections, then packs per-core per-item
    operands.  Phase 2 per macro-tile, all in transposed score layout:
        S^T[s,t] = sum_n kT[n,s] qT[n,t]      (PE, fp16)
        Sm^T = S^T * lT                       (3-path split, fp16 out)
        out[t,d] += Sm^T[s,t-chunk]^T-free @ v[s,d]  (PE, accumulate)
    The PV matmul uses Sm^T chunks as the stationary operand so the
    output lands in natural [t, d] layout with full 128-partition use
    (half the PE row count of the [d, t] formulation).
  - The score-mask multiplies are statically load-balanced across
    three paths (GPSIMD cannot read PSUM on TRN2): direct DVE fp32,
    ACT-copy + DVE fp16 (2x mode), and ACT-copy + Pool fp16; the last
    items avoid the slow Pool path to shorten the tail.  Scratch
    matmuls prime the PE p-state while the first DMAs are in flight.
  - All input DMAs issue on one queue in exact consumption order (the
    transfer engine is a single serialized resource at 360 GB/s, and
    per-DMA HWDGE setup costs ~625 ns, so both bytes and DMA count are
    minimized via fp16 + batched 2D-packed layouts).
  - Per-item outputs are staged in SBUF and flushed with a few batched
    DMAs; the host sums the per-panel partials (fp32).
"""

import json

import numpy as np

T = 8192
D_IN = 512
D_QK = 64
D_V = 64
NCORES = 8
PANEL = 512  # rows per t-panel
NITEMS = 17  # macro-tiles per core

# ---------------------------------------------------------------------------
# Workaround: the walrus build in this container accepts only ONE sync-wait
# per instruction, but Tile attaches several (e.g. to the tail Drain).  Split
# multi-wait instructions at the BIR-JSON level by inserting single-wait NoOps
# on the same engine immediately before the instruction.
# ---------------------------------------------------------------------------
_fix_installed = [False]
_split_counter = [0]


def _fix_bir_json(bir_json):
    m = json.loads(bir_json)
    for f in m.get("functions", []):
        for blk in f.get("blocks", []):
            new_insts = []
            for inst in blk.get("instructions", []):
                si = inst.get("sync_info") or {}
                waits = si.get("on_wait") or []
                if len(waits) > 1:
                    for w in waits[:-1]:
                        _split_counter[0] += 1
                        new_insts.append({
                            "name": f"I-waitsplit-{_split_counter[0]}",
                            "opcode": "NoOp",
                            "engine": inst.get("engine"),
                            "ins": [],
                            "outs": [],
                            "sync_info": {"on_wait": [w], "on_update": []},
                        })
                    si = dict(si)
                    si["on_wait"] = waits[-1:]
                    inst = dict(inst)
                    inst["sync_info"] = si
                new_insts.append(inst)
            blk["instructions"] = new_insts
    return json.dumps(m).encode()


def _install_bir_fix():
    if _fix_installed[0]:
        return
    _fix_installed[0] = True
    import concourse.bass_utils as bu
    import concourse.bass2jax as b2j

    orig = bu.compile_bir_kernel

    def patched(bir_json, tmpdir, neff_name="file.neff"):
        return orig(_fix_bir_json(bir_json), tmpdir, neff_name)

    bu.compile_bir_kernel = patched
    b2j.compile_bir_kernel = patched


# ---------------------------------------------------------------------------
# Per-core work-item list: (t0, s0) macro-tile origins, 17 per core.
# ---------------------------------------------------------------------------
def _core_items(c):
    """17 macro-tiles: positions 0/1 are the two diagonal tiles (uniform
    across cores), positions 2..16 the fifteen strictly-lower full tiles."""
    tA = 512 * c
    tB = T - 512 * (c + 1)
    items = [(tA, tA), (tB, tB)]
    items += [(tA, 512 * j) for j in range(c)]
    items += [(tB, 512 * j) for j in range(15 - c)]
    assert len(items) == NITEMS
    return items


# kq/vp DMA batching over items: 5 batches.
BATCHES = [(0, 2), (2, 6), (6, 10), (10, 14), (14, 17)]
DIAG_OFF = [0, 512, 896, 1152]  # prefix sums of widths 512,384,256,128

# PE p-state filler tuning (rows of scratch matmul work)
P1_FILL_PRE = 4
P2_FILL_PRE = 6
P2_FILL_STEP = 0
P2_MD = 3  # mult stage trails S stage
P2_BD = 1  # path-B second stage trails its ACT copy
P2_SMT_BUFS = 16
P2_MP = 7  # PV stage trails S stage
# lw full-tile DMA batching (indices into the 15 full tiles)
LW_BATCH = [(0, 2), (2, 4), (4, 6), (6, 8), (8, 10), (10, 11), (11, 12),
            (12, 13), (13, 14), (14, 15)]


# ---------------------------------------------------------------------------
# Bass kernel builders
# ---------------------------------------------------------------------------
def _build_phase1():
    import concourse.bass as bass
    import concourse.mybir as mybir
    from concourse.tile import TileContext

    f32 = mybir.dt.float32
    f16 = mybir.dt.float16

    nc = bass.Bass(target_bir_lowering=False, trn_type="TRN2")
    # host-pre-transposed, fp16, d-chunk-major x block: [128 d, dc*1024 + t]
    xTp = nc.dram_tensor("xTp", [128, 4096], f16, kind="ExternalInput")
    # packed weights: wq chunks (4*64) | wk chunks | wv chunks
    Wp = nc.dram_tensor("Wp", [128, 768], f16, kind="ExternalInput")
    qk_o = nc.dram_tensor("qk_o", [64, 2048], f16, kind="ExternalOutput")
    v_o = nc.dram_tensor("v_o", [128, 512], f16, kind="ExternalOutput")

    with TileContext(nc) as tc:
        with (
            tc.tile_pool(name="sb", bufs=1) as sb,
            tc.tile_pool(name="ps", bufs=1, space="PSUM") as ps,
        ):
            w = sb.tile([128, 768], f16, tag="w")
            nc.scalar.dma_start(w[:], Wp[:])
            # d-chunked x DMAs so the PE can start after the first chunk
            xt = {}
            for dc in range(4):
                xc = sb.tile([128, 1024], f16, tag=f"x{dc}", name=f"x{dc}")
                nc.sync.dma_start(xc[:], xTp[:, dc * 1024:(dc + 1) * 1024])
                for h in range(2):
                    xt[(dc, h)] = xc[:, h * 512:(h + 1) * 512]

            # PE p-state priming: the tensor engine clock ramps only under
            # sustained execution, so burn the cold period on scratch
            # matmuls while the first DMAs are in flight; real matmuls then
            # run at full clock.
            scr = sb.tile([128, 512], f16, tag="scr")
            nc.vector.memset(scr[:], 0.0)
            psF = ps.tile([128, 512], f32, tag="psF", name="psF")

            def filler(rows):
                nc.tensor.matmul(
                    psF[:, :rows],
                    scr[:, :128],
                    scr[:, :rows],
                    start=True,
                    stop=True,
                )

            for _ in range(P1_FILL_PRE):
                filler(512)

            # qT/kT: 4 psum groups (q/k x 2 sbuf-bank halves of t),
            # dc-outer order: all groups touch chunk dc before dc+1, so the
            # PE never waits on a chunk that is still in flight.
            pq = [
                ps.tile([64, 512], f32, tag=f"pq{j}", name=f"pq{j}")
                for j in range(4)
            ]
            for dc in range(4):
                for h in range(2):
                    for qk in range(2):
                        j = qk * 2 + h
                        wbase = qk * 256  # 0 -> Wq, 256 -> Wk
                        nc.tensor.matmul(
                            pq[j][:],
                            w[:, wbase + dc * 64:wbase + (dc + 1) * 64],
                            xt[(dc, h)][:],
                            start=(dc == 0),
                            stop=(dc == 3),
                        )
            # copies split between ACT and DVE; each qk half is flushed as
            # soon as its two copies land, on the queue that will not block
            # anything behind it (qk halves on sync, v halves on scalar).
            qk_st = sb.tile([64, 2048], f16, tag="qkst")
            for j in range(4):
                if j % 2 == 0:
                    nc.scalar.copy(qk_st[:, j * 512:(j + 1) * 512], pq[j][:])
                else:
                    nc.vector.tensor_copy(
                        qk_st[:, j * 512:(j + 1) * 512], pq[j][:]
                    )
                if j == 1:
                    nc.sync.dma_start(qk_o[:, 0:1024], qk_st[:, 0:1024])
            nc.sync.dma_start(qk_o[:, 1024:2048], qk_st[:, 1024:2048])

            # v in natural [t, d] layout: stationary x chunks, moving Wv.
            # Two psum banks of 4 t-chunks each: bank B accumulates while
            # bank A is drained in one bulk copy, so the PE never waits.
            v_st = sb.tile([128, 512], f16, tag="vst")
            for half in range(2):
                pvh = ps.tile([128, 256], f32, tag=f"pv{half}",
                              name=f"pv{half}")
                for tq in range(4):
                    tcn = half * 4 + tq
                    for dc in range(4):
                        nc.tensor.matmul(
                            pvh[:, tq * 64:(tq + 1) * 64],
                            xt[(dc, half)][:, tq * 128:(tq + 1) * 128],
                            w[:, 512 + dc * 64:512 + (dc + 1) * 64],
                            start=(dc == 0),
                            stop=(dc == 3),
                        )
                if half == 0:
                    nc.vector.tensor_copy(v_st[:, 0:256], pvh[:])
                    nc.sync.dma_start(v_o[:, 0:256], v_st[:, 0:256])
                else:
                    nc.scalar.copy(v_st[:, 256:512], pvh[:])
            nc.sync.dma_start(v_o[:, 256:512], v_st[:, 256:512])
    return nc


def _build_phase2():
    import concourse.bass as bass
    import concourse.mybir as mybir
    from concourse.tile import TileContext

    f32 = mybir.dt.float32
    f16 = mybir.dt.float16

    nc = bass.Bass(target_bir_lowering=False, trn_type="TRN2")
    # diag tiles, dense-packed lower-tri chunks, both items side by side
    lwd = nc.dram_tensor("lwd", [128, 2560], f16, kind="ExternalInput")
    lwp = nc.dram_tensor("lwp", [128, 15 * 2048], f16, kind="ExternalInput")
    # per item: kT tile (512) | qT tile (512), item-major columns
    kqp = nc.dram_tensor("kqp", [64, NITEMS * 1024], f16, kind="ExternalInput")
    # per item: v tile packed [128 s, sc*64 + d], item-major columns
    vp = nc.dram_tensor("vp", [128, NITEMS * 256], f16, kind="ExternalInput")
    # per item: out packed [128 t, tc*64 + d], item-major columns
    po = nc.dram_tensor("po", [128, NITEMS * 256], f16, kind="ExternalOutput")

    # Static load-balancer for the mask multiplies.  GPSIMD cannot touch
    # PSUM on TRN2, so the three legal paths are:
    #   A: DVE multiplies straight from PSUM (fp32 rate)
    #   B: ACT copies PSUM -> SBUF fp16, DVE multiplies all-fp16 (2x rate)
    #   C: ACT copies PSUM -> SBUF fp16, Pool multiplies (SBUF-only ok)
    eng_t = {"d": 0.0, "a": 0.0, "g": 0.0}

    def pick_path(width, allow_pool=True):
        cA = width * 1.0417 + 125.0
        cBa = width * 0.833 + 230.0
        cBd = width * 0.52 + 60.0
        cCg = width * 1.984 + 30.0
        endA = max(eng_t["d"] + cA, eng_t["a"], eng_t["g"])
        endB = max(eng_t["d"] + cBd, eng_t["a"] + cBa, eng_t["g"])
        endC = max(eng_t["d"], eng_t["a"] + cBa, eng_t["g"] + cCg)
        if not allow_pool:
            endC = float("inf")
        best = min(endA, endB, endC)
        if best == endA:
            eng_t["d"] += cA
            return "A"
        if best == endB:
            eng_t["d"] += cBd
            eng_t["a"] += cBa
            return "B"
        eng_t["a"] += cBa
        eng_t["g"] += cCg
        return "C"

    with TileContext(nc) as tc:
        with (
            tc.tile_pool(name="lw", bufs=1) as lwpool,
            tc.tile_pool(name="ops", bufs=1) as ops,
            tc.tile_pool(name="smt", bufs=8) as smtp,
            tc.tile_pool(name="stage", bufs=1) as stg,
            tc.tile_pool(name="psS", bufs=3, space="PSUM") as psS,
            tc.tile_pool(name="psO", bufs=2, space="PSUM") as psO,
        ):
            # ---- input DMA plan: lw stream on sync, kq/vp stream on scalar
            lwdt = lwpool.tile([128, 2560], f16, tag="lwd")
            kqt, vpt = [], []
            for b, (i0, i1) in enumerate(BATCHES):
                nb = i1 - i0
                kqt.append(
                    ops.tile([64, nb * 1024], f16, tag=f"kq{b}", name=f"kq{b}")
                )
                vpt.append(
                    ops.tile([128, nb * 256], f16, tag=f"vp{b}", name=f"vp{b}")
                )
            lwbt = []
            for j, (j0, j1) in enumerate(LW_BATCH):
                lwbt.append(
                    lwpool.tile(
                        [128, (j1 - j0) * 2048], f16, tag=f"lwb{j}",
                        name=f"lwb{j}",
                    )
                )

            def lw_of(i):
                """SBUF slice holding l tile i (full tiles only, i >= 2)."""
                for j, (j0, j1) in enumerate(LW_BATCH):
                    if j0 <= i - 2 < j1:
                        return lwbt[j][:, (i - 2 - j0) * 2048:(i - 1 - j0) * 2048]
                raise AssertionError

            # Issue ALL input DMAs on the sync queue in exact consumption
            # order: the transfer engine serves one queue in program order,
            # so the kq/vp batches land just-in-time for the S stage without
            # preempting the lw stream that feeds the mask multiplies.
            def kv_batch(b):
                i0, i1 = BATCHES[b]
                nc.sync.dma_start(kqt[b][:], kqp[:, i0 * 1024:i1 * 1024])
                nc.sync.dma_start(vpt[b][:], vp[:, i0 * 256:i1 * 256])

            nc.sync.dma_start(lwdt[:, 0:1280], lwd[:, 0:1280])
            kv_batch(0)
            nc.sync.dma_start(lwdt[:, 1280:2560], lwd[:, 1280:2560])
            kv_batch(1)
            for j, (j0, j1) in enumerate(LW_BATCH):
                if j in (2, 4, 6):
                    kv_batch(2 + (j - 2) // 2)
                nc.sync.dma_start(lwbt[j][:], lwp[:, j0 * 2048:j1 * 2048])

            stA = stg.tile([128, 8 * 256], f16, tag="stA")
            stB = stg.tile([128, 9 * 256], f16, tag="stB")

            # PE p-state priming (see phase 1): scratch matmuls ramp the
            # clock before real work arrives and plug feed gaps after.
            scr = stg.tile([128, 512], f16, tag="scr")
            nc.vector.memset(scr[:], 0.0)
            psF = psO.tile([128, 512], f32, tag="psF", name="psF", bufs=1)

            def filler(rows):
                nc.tensor.matmul(
                    psF[:, :rows],
                    scr[:, :128],
                    scr[:, :rows],
                    start=True,
                    stop=True,
                )

            for _ in range(P2_FILL_PRE):
                filler(512)

            def batch_of(i):
                for b, (i0, i1) in enumerate(BATCHES):
                    if i0 <= i < i1:
                        return b, i - i0
                raise AssertionError

            def geom(i, sc):
                if i < 2:
                    return 512 - 128 * sc, 128 * sc, DIAG_OFF[sc]
                return 512, 0, 512 * sc

            # Software pipeline over 68 (item, sc) stages, three decoupled
            # stage trails: the S matmul for stage s runs at step s, its
            # mask multiply (DVE/Pool) at s+MD, its PV matmuls at s+MP.
            # The deep PV trail keeps the PE queue head from ever waiting
            # on a mult result; scratch fillers plug residual feed gaps so
            # the PE p-state stays at full clock.
            MD = P2_MD
            MP = P2_MP
            NST = NITEMS * 4
            s_tiles = {}
            c_tiles = {}
            m_tiles = {}
            paths = {}
            out_ps = None

            def stage_ops(s):
                i, sc = divmod(s, 4)
                b, bo = batch_of(i)
                return (
                    i,
                    sc,
                    kqt[b][:, bo * 1024:(bo + 1) * 1024],
                    vpt[b][:, bo * 256:(bo + 1) * 256],
                    lwdt[:, i * 1280:(i + 1) * 1280] if i < 2 else lw_of(i),
                )

            for step in range(NST + MP):
                if step < NST:
                    i, sc, kq, vw, lw = stage_ops(step)
                    w, t0, _ = geom(i, sc)
                    s_ps = psS.tile([128, 512], f32, tag="S", bufs=MD + 2)
                    nc.tensor.matmul(
                        s_ps[:, :w],
                        kq[:, sc * 128:(sc + 1) * 128],
                        kq[:, 512 + t0:1024],
                        start=True,
                        stop=True,
                    )
                    s_tiles[step] = s_ps
                if MD <= step < NST + MD:
                    m = step - MD
                    i, sc, kq, vw, lw = stage_ops(m)
                    w, t0, off = geom(i, sc)
                    s_ps = s_tiles.pop(m)
                    smt = smtp.tile([128, 512], f16, tag="smt", bufs=P2_SMT_BUFS)
                    paths[m] = pick_path(w, allow_pool=(m < NST - 8))
                    if paths[m] == "A":
                        nc.vector.tensor_mul(
                            smt[:, :w], s_ps[:, :w], lw[:, off:off + w]
                        )
                    else:
                        sc16 = smtp.tile([128, 512], f16, tag="sc16", bufs=6)
                        nc.scalar.copy(sc16[:, :w], s_ps[:, :w])
                        c_tiles[m] = (sc16, lw, off, w)
                    m_tiles[m] = smt
                if MD + P2_BD <= step < NST + MD + P2_BD:
                    m = step - MD - P2_BD
                    if paths[m] != "A":
                        sc16, lw, off, w = c_tiles.pop(m)
                        smt = m_tiles[m]
                        eng = nc.vector if paths[m] == "B" else nc.gpsimd
                        eng.tensor_mul(
                            smt[:, :w], sc16[:, :w], lw[:, off:off + w]
                        )
                if step >= MP and (step - MP) % 4 == 3:
                    # whole-item PV block, tcn-major so each psum
                    # accumulation group closes before the next opens
                    i = (step - MP) // 4
                    _, _, kq, vw, lw = stage_ops(4 * i)
                    diag = i < 2
                    out_ps = psO.tile([128, 256], f32, tag="out")
                    smts = [m_tiles.pop(4 * i + sc) for sc in range(4)]
                    # PV: stationary = Sm^T t-chunk, moving = v chunk ->
                    # out[t, d] accumulated over s-chunks
                    for tcn in range(4):
                        for sc in range(0, (tcn + 1) if diag else 4):
                            col0 = (tcn - sc) * 128 if diag else tcn * 128
                            nc.tensor.matmul(
                                out_ps[:, tcn * 64:(tcn + 1) * 64],
                                smts[sc][:, col0:col0 + 128],
                                vw[:, sc * 64:(sc + 1) * 64],
                                start=(sc == 0),
                                stop=(sc == (tcn if diag else 3)),
                            )
                    if True:
                        st, o0 = (stA, i) if i < 8 else (stB, i - 8)
                        if eng_t["a"] <= eng_t["d"]:
                            eng_t["a"] += 360.0
                            nc.scalar.copy(
                                st[:, o0 * 256:(o0 + 1) * 256], out_ps[:]
                            )
                        else:
                            eng_t["d"] += 400.0
                            nc.vector.tensor_copy(
                                st[:, o0 * 256:(o0 + 1) * 256], out_ps[:]
                            )
                        if i < 8:
                            if i == 5:
                                nc.sync.dma_start(po[:, 0:6 * 256],
                                                  stA[:, 0:6 * 256])
                            if i == 7:
                                nc.sync.dma_start(po[:, 6 * 256:8 * 256],
                                                  stA[:, 6 * 256:])
                        else:
                            j = i - 8
                            if j == 5:
                                nc.sync.dma_start(
                                    po[:, 8 * 256:14 * 256], stB[:, 0:6 * 256]
                                )
                            if j == 7:
                                nc.sync.dma_start(
                                    po[:, 14 * 256:16 * 256],
                                    stB[:, 6 * 256:8 * 256],
                                )
                if step < NST and P2_FILL_STEP:
                    filler(P2_FILL_STEP)
            nc.sync.dma_start(po[:, 16 * 256:NITEMS * 256],
                              stB[:, 8 * 256:])
    return nc


_nc_cache = {}


def _get_nc(which):
    if which not in _nc_cache:
        _nc_cache[which] = _build_phase1() if which == 1 else _build_phase2()
    return _nc_cache[which]


# ---------------------------------------------------------------------------
# Host-side packing helpers (pure data movement + dtype cast)
# ---------------------------------------------------------------------------
def _pack_chunks(a, nchunk, rows):
    """[nchunk*rows, w] -> [rows, nchunk*w] with chunk-major free dim."""
    w = a.shape[1]
    return np.ascontiguousarray(
        a.reshape(nchunk, rows, w).transpose(1, 0, 2).reshape(rows, nchunk * w)
    )


def kernel(x, Wq, Wk, Wv, l):
    _install_bir_fix()
    from concourse import bass_utils

    x = np.asarray(x, dtype=np.float32)
    l = np.asarray(l, dtype=np.float32)

    core_ids = list(range(NCORES))

    # ---------------- Phase 1: per-core projections -----------------------
    Wp = np.concatenate(
        [
            _pack_chunks(np.asarray(wm, dtype=np.float16), 4, 128)
            for wm in (Wq, Wk, Wv)
        ],
        axis=1,
    )  # [128, 768]
    in1 = []
    panels = []
    for c in range(NCORES):
        tA = 512 * c
        tB = T - 512 * (c + 1)
        panels.append((tA, tB))
        xcat = np.concatenate([x[tA:tA + 512], x[tB:tB + 512]], axis=0)
        xT = np.ascontiguousarray(xcat.T).astype(np.float16)  # [512, 1024]
        xTp = _pack_chunks(xT, 4, 128)  # [128, 4096]
        in1.append({"xTp": xTp, "Wp": Wp})

    res1 = bass_utils.run_bass_kernel_spmd(_get_nc(1), in1, core_ids=core_ids)

    qT_full = np.empty((64, T), dtype=np.float16)
    kT_full = np.empty((64, T), dtype=np.float16)
    v_full = np.empty((T, 64), dtype=np.float16)
    for c in range(NCORES):
        tA, tB = panels[c]
        r = res1.results[c]
        qk = r["qk_o"]  # [64, 2048]: qA | qB | kA | kB
        qT_full[:, tA:tA + 512] = qk[:, 0:512]
        qT_full[:, tB:tB + 512] = qk[:, 512:1024]
        kT_full[:, tA:tA + 512] = qk[:, 1024:1536]
        kT_full[:, tB:tB + 512] = qk[:, 1536:2048]
        vup = r["v_o"].reshape(128, 8, 64).transpose(1, 0, 2).reshape(1024, 64)
        v_full[tA:tA + 512] = vup[:512]
        v_full[tB:tB + 512] = vup[512:]

    # ---------------- Phase 2: masked scores + PV -------------------------
    in2 = []
    for c in range(NCORES):
        items = _core_items(c)
        lwd = np.empty((128, 2560), dtype=np.float16)
        lwp = np.empty((128, 15 * 2048), dtype=np.float16)
        kqp = np.empty((64, NITEMS * 1024), dtype=np.float16)
        vpk = np.empty((128, NITEMS * 256), dtype=np.float16)
        for i, (t0, s0) in enumerate(items):
            lt = l[t0:t0 + 512, s0:s0 + 512]
            if i < 2:
                lT = np.tril(lt).T.astype(np.float16)  # [512 s, 512 t]
                for sc in range(4):
                    w = 512 - 128 * sc
                    o = i * 1280 + DIAG_OFF[sc]
                    lwd[:, o:o + w] = lT[128 * sc:128 * (sc + 1), 128 * sc:512]
            else:
                lT = lt.T.astype(np.float16)
                lwp[:, (i - 2) * 2048:(i - 1) * 2048] = (
                    lT.reshape(4, 128, 512).transpose(1, 0, 2).reshape(128, 2048)
                )
            kqp[:, i * 1024:i * 1024 + 512] = kT_full[:, s0:s0 + 512]
            kqp[:, i * 1024 + 512:(i + 1) * 1024] = qT_full[:, t0:t0 + 512]
            vpk[:, i * 256:(i + 1) * 256] = (
                v_full[s0:s0 + 512]
                .reshape(4, 128, 64)
                .transpose(1, 0, 2)
                .reshape(128, 256)
            )
        in2.append({"lwd": lwd, "lwp": lwp, "kqp": kqp, "vp": vpk})

    res2 = bass_utils.run_bass_kernel_spmd(_get_nc(2), in2, core_ids=core_ids)

    out = np.empty((T, 64), dtype=np.float32)
    for c in range(NCORES):
        tA, tB = panels[c]
        p = res2.results[c]["po"].astype(np.float32)  # [128, 17*256]
        # unpack per item: [128 t, tc*64 + d] -> [512 t, 64 d]
        pit = (
            p.reshape(128, NITEMS, 4, 64)
            .transpose(1, 2, 0, 3)
            .reshape(NITEMS, 512, 64)
        )
        # item 0 = diag A, item 1 = diag B, 2..2+c full A, rest full B
        out[tA:tA + 512] = pit[0] + pit[2:2 + c].sum(axis=0)
        out[tB:tB + 512] = pit[1] + pit[2 + c:].sum(axis=0)
    return out
